# revision 1
# baseline (speedup 1.0000x reference)
"""Trainium2 Bass kernel for nn_ConvAlignLoss (8-core data parallel).

Self-contained: hardcodes shapes; imports concourse from /opt/trn_rl_repo.

Per core (R=64 rows):
  loss_astf partial: sum((pred-true)^2)
  conv = irfft16384(fft(pred) * conj(fft(egf_pad)))[:14337]  (2-stage matmul FFT)
  cc   = irfft32768(fft(conv_pad) * conj(fft(target_pad)))
  shift = mapped masked argmax of cc (== reference argmax over n=28673)
  loss_conv partial: sum((conv[(7040+i+shift) % 14337] - target[7040+i])^2)
Host combines the 8 cores' (sum_astf, sum_conv) into the scalar losses.

FFT structure (N = 128*N2):
  FWD:  D[t1,t2]=x[N2*t1+t2]; A[t2,f1]=sum_t1 D*W1 (data-stationary matmul);
        B=A*tw; Z[f2,f1]=sum_t2 W2[t2,f2]*B.   Z2d[f2,f1] == X[f1+128*f2]
  INV:  G[f1,t2]=sum_f2 S2d[f2,f1]*V2[f2,t2] (S stationary); H=G*itw;
        x2d[t1,t2]=(1/N) Re(sum_f1 V1[f1,t1]*H[f1,t2])
"""
import sys

sys.path.insert(0, "/opt/trn_rl_repo")

import numpy as np
import concourse.bass as bass
import concourse.bacc as bacc
import concourse.mybir as mybir
from concourse import tile

F32 = mybir.dt.float32
BF16 = mybir.dt.bfloat16
I32 = mybir.dt.int32
AT = mybir.AluOpType
AX = mybir.AxisListType

R = 64
NCORES = 8
L1, L2 = 16384, 2048
CONV_LEN = L1 - L2 + 1      # 14337
N_A, N_B = 16384, 32768
GAP_LO, GAP_HI = CONV_LEN, N_B - CONV_LEN + 1   # gap [14337, 18432)
CROP = 256
START0 = (CONV_LEN - CROP) // 2                 # 7040
PITCH = 14720
BIGL = float(2 ** 23)


def _dft(n, sign):
    k = np.arange(n)
    return np.exp(sign * 2j * np.pi * np.outer(k, k) / n)


def make_consts():
    c = {}

    def put(name, arr, dt=np.float32):
        c[name] = np.ascontiguousarray(np.asarray(arr, np.float64)).astype(dt)

    W1 = _dft(128, -1)
    put("W1r", W1.real); put("W1i", W1.imag); put("nW1i", -W1.imag)
    twA = np.exp(-2j * np.pi * np.outer(np.arange(128), np.arange(128)) / N_A)
    put("twAr", twA.real); put("twAi", twA.imag)
    V2A = _dft(128, +1)
    put("V2Ar", V2A.real); put("V2Ai", V2A.imag); put("nV2Ai", -V2A.imag)
    itwA = np.exp(2j * np.pi * np.outer(np.arange(128), np.arange(128)) / N_A)
    put("itwAr", itwA.real); put("itwAi", itwA.imag)
    V1A = _dft(128, +1) / N_A
    put("V1Ar", V1A.real); put("nV1Ai", -V1A.imag)

    W2B = _dft(256, -1)          # [t2, f2]
    for a in range(2):
        for b in range(2):
            blk = W2B[a * 128:(a + 1) * 128, b * 128:(b + 1) * 128]
            put(f"W2Br{a}{b}", blk.real)
            put(f"W2Bi{a}{b}", blk.imag)
            put(f"nW2Bi{a}{b}", -blk.imag)
    twB = np.exp(-2j * np.pi * np.outer(np.arange(256), np.arange(128)) / N_B)
    for a in range(2):
        put(f"twBr{a}", twB.real[a * 128:(a + 1) * 128])
        put(f"twBi{a}", twB.imag[a * 128:(a + 1) * 128])
    V2B = _dft(256, +1)          # [f2, t2]
    for a in range(2):
        blk = V2B[a * 128:(a + 1) * 128, :]
        put(f"V2Br{a}", blk.real)
        put(f"V2Bi{a}", blk.imag)
        put(f"nV2Bi{a}", -blk.imag)
    itwB = np.exp(2j * np.pi * np.outer(np.arange(128), np.arange(256)) / N_B)
    put("itwBr", itwB.real); put("itwBi", itwB.imag)
    V1B = _dft(128, +1) / N_B
    put("V1Br", V1B.real); put("nV1Bi", -V1B.imag)

    put("ident", np.eye(128))
    put("ones1x128", np.ones((1, 128)))
    put("ones128", np.ones((128, 1)))
    put("ones64", np.ones((64, 1)))

    j = np.arange(128)[:, None] * 256 + np.arange(256)[None, :]   # [t1, t2]
    gap = (j >= GAP_LO) & (j < GAP_HI)
    put("maskB", np.where(gap, -1e30, 0.0))
    shiftval = np.where(j <= CONV_LEN - 1, j - (CONV_LEN - 1), j - GAP_HI + 1)
    put("shvB", np.where(gap, 0.0, shiftval - BIGL))
    put("winidx", np.arange(R)[:, None] * PITCH
        + np.arange(CROP)[None, :])                               # [64, 256]
    return c


def _b3(ap, n, inner):
    """[128, inner] const AP -> [128, n, inner] broadcast over middle dim."""
    return ap.rearrange("p (a b) -> p a b", a=1).to_broadcast([128, n, inner])


def _cmul_psum(nc, pool, tag, outr, outi, pr, pi, twr, twi, inner, n):
    """(outr + i outi) = (pr + i pi) * (twr + i twi); p* in PSUM, tw const APs
    broadcast over n blocks of `inner`. outr/outi are SBUF APs [128, n*inner]."""
    tmp = pool.tile([128, n * inner], F32, tag="twtmp", name="twtmp")
    orv = outr.rearrange("p (a b) -> p a b", b=inner)
    oiv = outi.rearrange("p (a b) -> p a b", b=inner)
    prv = pr.rearrange("p (a b) -> p a b", b=inner)
    piv = pi.rearrange("p (a b) -> p a b", b=inner)
    tv = tmp[:].rearrange("p (a b) -> p a b", b=inner)
    nc.vector.tensor_tensor(orv, prv, twr, op=AT.mult)
    nc.vector.tensor_tensor(tv, piv, twi, op=AT.mult)
    nc.vector.tensor_tensor(orv, orv, tv, op=AT.subtract)
    nc.vector.tensor_tensor(oiv, prv, twi, op=AT.mult)
    nc.vector.tensor_tensor(tv, piv, twr, op=AT.mult)
    nc.vector.tensor_tensor(oiv, oiv, tv, op=AT.add)


def _mm_const_names():
    s = {"W1r", "W1i", "nW1i", "V2Ar", "V2Ai", "nV2Ai", "V1Ar", "nV1Ai",
         "itwAr", "itwAi", "twAr", "twAi", "itwBr", "itwBi", "V1Br",
         "nV1Bi", "ident"}
    s |= {f"W2Br{a}{b}" for a in range(2) for b in range(2)}
    s |= {f"W2Bi{a}{b}" for a in range(2) for b in range(2)}
    s |= {f"nW2Bi{a}{b}" for a in range(2) for b in range(2)}
    s |= {f"twBr{a}" for a in range(2)} | {f"twBi{a}" for a in range(2)}
    s |= {f"V2Br{a}" for a in range(2)} | {f"V2Bi{a}" for a in range(2)}
    s |= {f"nV2Bi{a}" for a in range(2)}
    return s


def build_nc(cdt=F32, rows=R, rbb=8, rb2=4):
    nc = bacc.Bacc("TRN2", target_bir_lowering=False, debug=False,
                   num_devices=NCORES)
    consts = make_consts()

    pred = nc.dram_tensor("pred", [rows, L1], F32, kind="ExternalInput")
    true_ = nc.dram_tensor("true", [rows, L1], F32, kind="ExternalInput")
    egf = nc.dram_tensor("egf", [rows, L2], F32, kind="ExternalInput")
    target = nc.dram_tensor("target", [rows, CONV_LEN], F32,
                            kind="ExternalInput")
    out = nc.dram_tensor("out", [1, 2], F32, kind="ExternalOutput")
    scratch = nc.dram_tensor("scratch", [rows, PITCH], F32)

    MM_CONST = _mm_const_names()

    cdram = {}
    for name, arr in consts.items():
        cdt_n = cdt if name in MM_CONST else F32
        cdram[name] = nc.dram_tensor(name, list(arr.shape), cdt_n,
                                     kind="ExternalInput")

    nb1, nb2 = rows // rbb, rows // rb2

    with tile.TileContext(nc) as tc:
        with (
            tc.tile_pool(name="consts", bufs=1) as cpool,
            tc.tile_pool(name="keep", bufs=1) as kpool,
            tc.tile_pool(name="ps", bufs=2, space="PSUM") as pp,
        ):
            cs = {}
            for name, arr in consts.items():
                dt = cdt if name in MM_CONST else F32
                t = cpool.tile(list(arr.shape), dt, tag=f"c_{name}", name=f"c_{name}")
                nc.sync.dma_start(t[:], cdram[name][:])
                cs[name] = t

            allmax = kpool.tile([128, rows], F32, tag="allmax", name="allmax")
            allmin = kpool.tile([128, rows], F32, tag="allmin", name="allmin")
            ccm_all = kpool.tile([128, rows * 256], BF16, tag="ccm", name="ccm")
            astf_acc = kpool.tile([128, 8], F32, tag="astfacc", name="astfacc")
            shifts = kpool.tile([rows, 1], F32, tag="shifts", name="shifts")
            outt = kpool.tile([1, 2], F32, tag="outt", name="outt")

            # ---------------- A) astf ----------------
            predf = pred.ap().rearrange("r l -> (r l)").rearrange(
                "(p f) -> p f", p=128)
            truef = true_.ap().rearrange("r l -> (r l)").rearrange(
                "(p f) -> p f", p=128)
            fch = rows * L1 // 128 // 8
            with tc.tile_pool(name="astf", bufs=2) as apool:
                for i in range(8):
                    tp = apool.tile([128, fch], F32, tag="ap", name="ap")
                    tt = apool.tile([128, fch], F32, tag="at", name="at")
                    sl = bass.ts(i, fch)
                    nc.sync.dma_start(tp[:], predf[:, sl])
                    nc.sync.dma_start(tt[:], truef[:, sl])
                    nc.vector.tensor_tensor(tt[:], tp[:], tt[:], op=AT.subtract)
                    nc.vector.scalar_tensor_tensor(
                        tp[:], tt[:], 1.0, tt[:], op0=AT.bypass, op1=AT.mult,
                        accum_out=astf_acc[:, i:i + 1])

            # ---------------- B) 16K level ----------------
            with tc.tile_pool(name="p16", bufs=1) as dp:
                for b in range(nb1):
                    r0 = b * rbb
                    Dp = dp.tile([128, rbb * 128], cdt, tag="Dp", name="Dp")
                    De = dp.tile([128, rbb * 128], cdt, tag="De", name="De")
                    nc.scalar.memzero(De[:])
                    if cdt == F32:
                        for q in range(rbb):
                            r = r0 + q
                            nc.sync.dma_start(
                                Dp[:, bass.ts(q, 128)],
                                pred[r, :].rearrange("(a b) -> a b", a=128))
                            nc.sync.dma_start(
                                De[:16, bass.ts(q, 128)],
                                egf[r, :].rearrange("(a b) -> a b", a=16))
                    else:
                        Dst = dp.tile([128, rbb * 128], F32, tag="Dst", name="Dst")
                        Est = dp.tile([16, rbb * 128], F32, tag="Est", name="Est")
                        for q in range(rbb):
                            r = r0 + q
                            nc.sync.dma_start(
                                Dst[:, bass.ts(q, 128)],
                                pred[r, :].rearrange("(a b) -> a b", a=128))
                            nc.sync.dma_start(
                                Est[:16, bass.ts(q, 128)],
                                egf[r, :].rearrange("(a b) -> a b", a=16))
                        nc.scalar.copy(Dp[:], Dst[:])
                        nc.scalar.copy(De[:16, :], Est[:16, :])

                    Bs = {k: dp.tile([128, rbb * 128], cdt, tag=f"B{k}", name=f"B{k}")
                          for k in ("pr", "pi", "er", "ei")}
                    for g in range(rbb // 4):
                        gsl = bass.ts(g, 512)
                        for inp, D in (("p", Dp), ("e", De)):
                            pa = pp.tile([128, 512], F32, tag="st1", name="st1")
                            pai = pp.tile([128, 512], F32, tag="st1", name="st1")
                            for q in range(4):
                                qq = g * 4 + q
                                sl, osl = bass.ts(qq, 128), bass.ts(q, 128)
                                nc.tensor.matmul(pa[:, osl], lhsT=D[:, sl],
                                                 rhs=cs["W1r"][:],
                                                 start=True, stop=True)
                                nc.tensor.matmul(pai[:, osl], lhsT=D[:, sl],
                                                 rhs=cs["W1i"][:],
                                                 start=True, stop=True)
                            _cmul_psum(nc, dp, "tw",
                                       Bs[inp + "r"][:, gsl], Bs[inp + "i"][:, gsl],
                                       pa[:], pai[:],
                                       _b3(cs["twAr"][:], 4, 128),
                                       _b3(cs["twAi"][:], 4, 128), 128, 4)

                    Zs = {k: dp.tile([128, rbb * 128], cdt, tag=f"Z{k}", name=f"Z{k}")
                          for k in ("pr", "pi", "er", "ei")}
                    for g in range(rbb // 4):
                        gsl = bass.ts(g, 512)
                        for inp in ("p", "e"):
                            br, bi = Bs[inp + "r"], Bs[inp + "i"]
                            pzr = pp.tile([128, 512], F32, tag="st2", name="st2")
                            pzi = pp.tile([128, 512], F32, tag="st2", name="st2")
                            nc.tensor.matmul(pzr[:], lhsT=cs["W1r"][:],
                                             rhs=br[:, gsl], start=True, stop=False)
                            nc.tensor.matmul(pzr[:], lhsT=cs["nW1i"][:],
                                             rhs=bi[:, gsl], start=False, stop=True)
                            nc.tensor.matmul(pzi[:], lhsT=cs["W1i"][:],
                                             rhs=br[:, gsl], start=True, stop=False)
                            nc.tensor.matmul(pzi[:], lhsT=cs["W1r"][:],
                                             rhs=bi[:, gsl], start=False, stop=True)
                            nc.scalar.copy(Zs[inp + "r"][:, gsl], pzr[:])
                            nc.scalar.copy(Zs[inp + "i"][:, gsl], pzi[:])

                    Sr = dp.tile([128, rbb * 128], cdt, tag="Sr", name="Sr")
                    Si = dp.tile([128, rbb * 128], cdt, tag="Si", name="Si")
                    tmpb = dp.tile([128, rbb * 128], F32, tag="tmpbig", name="tmpbig")
                    nc.vector.tensor_tensor(Sr[:], Zs["pr"][:], Zs["er"][:], op=AT.mult)
                    nc.vector.tensor_tensor(tmpb[:], Zs["pi"][:], Zs["ei"][:], op=AT.mult)
                    nc.vector.tensor_tensor(Sr[:], Sr[:], tmpb[:], op=AT.add)
                    nc.vector.tensor_tensor(Si[:], Zs["pi"][:], Zs["er"][:], op=AT.mult)
                    nc.vector.tensor_tensor(tmpb[:], Zs["pr"][:], Zs["ei"][:], op=AT.mult)
                    nc.vector.tensor_tensor(Si[:], Si[:], tmpb[:], op=AT.subtract)

                    Hr = dp.tile([128, rbb * 128], cdt, tag="Hr", name="Hr")
                    Hi = dp.tile([128, rbb * 128], cdt, tag="Hi", name="Hi")
                    for g in range(rbb // 4):
                        gsl = bass.ts(g, 512)
                        pgr = pp.tile([128, 512], F32, tag="inv", name="inv")
                        pgi = pp.tile([128, 512], F32, tag="inv", name="inv")
                        for q in range(4):
                            qq = g * 4 + q
                            sl, osl = bass.ts(qq, 128), bass.ts(q, 128)
                            nc.tensor.matmul(pgr[:, osl], lhsT=Sr[:, sl],
                                             rhs=cs["V2Ar"][:], start=True, stop=False)
                            nc.tensor.matmul(pgr[:, osl], lhsT=Si[:, sl],
                                             rhs=cs["nV2Ai"][:], start=False, stop=True)
                            nc.tensor.matmul(pgi[:, osl], lhsT=Sr[:, sl],
                                             rhs=cs["V2Ai"][:], start=True, stop=False)
                            nc.tensor.matmul(pgi[:, osl], lhsT=Si[:, sl],
                                             rhs=cs["V2Ar"][:], start=False, stop=True)
                        _cmul_psum(nc, dp, "tw", Hr[:, gsl], Hi[:, gsl],
                                   pgr[:], pgi[:],
                                   _b3(cs["itwAr"][:], 4, 128),
                                   _b3(cs["itwAi"][:], 4, 128), 128, 4)

                    convSB = dp.tile([128, rbb * 128], F32, tag="convSB", name="convSB")
                    for g in range(rbb // 4):
                        gsl = bass.ts(g, 512)
                        pc = pp.tile([128, 512], F32, tag="cc", name="cc")
                        nc.tensor.matmul(pc[:], lhsT=cs["V1Ar"][:],
                                         rhs=Hr[:, gsl], start=True, stop=False)
                        nc.tensor.matmul(pc[:], lhsT=cs["nV1Ai"][:],
                                         rhs=Hi[:, gsl], start=False, stop=True)
                        nc.scalar.copy(convSB[:, gsl], pc[:])

                    for q in range(rbb):
                        r = r0 + q
                        csl = bass.ts(q, 128)
                        nc.sync.dma_start(
                            scratch[r, 0:14336].rearrange("(a b) -> a b", a=112),
                            convSB[0:112, csl])
                        nc.sync.dma_start(
                            scratch[r, 14336:14337].rearrange("(a b) -> a b", a=1),
                            convSB[112:113, q * 128:q * 128 + 1])
                        nc.sync.dma_start(
                            scratch[r, 14337:14593].rearrange("(a b) -> a b", a=2),
                            convSB[0:2, csl])

            # ---------------- C) 32K level ----------------
            with tc.tile_pool(name="p32", bufs=1) as dp:
                for b in range(nb2):
                    r0 = b * rb2
                    D2c = dp.tile([128, rb2 * 256], cdt, tag="D2c", name="D2c")
                    D2t = dp.tile([128, rb2 * 256], cdt, tag="D2t", name="D2t")
                    nc.scalar.memzero(D2c[:])
                    nc.scalar.memzero(D2t[:])
                    if cdt == F32:
                        tgc, tgt_ = D2c, D2t
                    else:
                        tgc = dp.tile([128, rb2 * 256], F32, tag="D2cs", name="D2cs")
                        tgt_ = dp.tile([128, rb2 * 256], F32, tag="D2ts", name="D2ts")
                        nc.scalar.memzero(tgc[:])
                        nc.scalar.memzero(tgt_[:])
                    for q in range(rb2):
                        r = r0 + q
                        sl = bass.ts(q, 256)
                        nc.sync.dma_start(
                            tgc[0:56, sl],
                            scratch[r, 0:14336].rearrange("(a b) -> a b", a=56))
                        nc.sync.dma_start(
                            tgc[56:57, q * 256:q * 256 + 1],
                            scratch[r, 14336:14337].rearrange("(a b) -> a b", a=1))
                        nc.sync.dma_start(
                            tgt_[0:56, sl],
                            target[r, 0:14336].rearrange("(a b) -> a b", a=56))
                        nc.sync.dma_start(
                            tgt_[56:57, q * 256:q * 256 + 1],
                            target[r, 14336:14337].rearrange("(a b) -> a b", a=1))
                    if cdt != F32:
                        nc.scalar.copy(D2c[0:57, :], tgc[0:57, :])
                        nc.scalar.copy(D2t[0:57, :], tgt_[0:57, :])

                    B2 = {}
                    for c in range(2):
                        for inp, D in (("c", D2c), ("t", D2t)):
                            br = dp.tile([128, rb2 * 128], cdt, tag=f"B2r{c}{inp}", name=f"B2r{c}{inp}")
                            bi = dp.tile([128, rb2 * 128], cdt, tag=f"B2i{c}{inp}", name=f"B2i{c}{inp}")
                            pa = pp.tile([128, rb2 * 128], F32, tag="st1", name="st1")
                            pai = pp.tile([128, rb2 * 128], F32, tag="st1", name="st1")
                            for q in range(rb2):
                                dsl = slice(q * 256 + c * 128,
                                            q * 256 + c * 128 + 128)
                                osl = bass.ts(q, 128)
                                nc.tensor.matmul(pa[:, osl], lhsT=D[:, dsl],
                                                 rhs=cs["W1r"][:],
                                                 start=True, stop=True)
                                nc.tensor.matmul(pai[:, osl], lhsT=D[:, dsl],
                                                 rhs=cs["W1i"][:],
                                                 start=True, stop=True)
                            _cmul_psum(nc, dp, "tw", br[:], bi[:], pa[:], pai[:],
                                       _b3(cs[f"twBr{c}"][:], rb2, 128),
                                       _b3(cs[f"twBi{c}"][:], rb2, 128), 128, rb2)
                            B2[(c, inp)] = (br, bi)

                    Z2 = {}
                    for inp in ("c", "t"):
                        for f2c in range(2):
                            zr = dp.tile([128, rb2 * 128], cdt, tag=f"Z2r{inp}{f2c}", name=f"Z2r{inp}{f2c}")
                            zi = dp.tile([128, rb2 * 128], cdt, tag=f"Z2i{inp}{f2c}", name=f"Z2i{inp}{f2c}")
                            pzr = pp.tile([128, rb2 * 128], F32, tag="st2", name="st2")
                            pzi = pp.tile([128, rb2 * 128], F32, tag="st2", name="st2")
                            for t2c in range(2):
                                br, bi = B2[(t2c, inp)]
                                nc.tensor.matmul(pzr[:], lhsT=cs[f"W2Br{t2c}{f2c}"][:],
                                                 rhs=br[:], start=(t2c == 0), stop=False)
                                nc.tensor.matmul(pzr[:], lhsT=cs[f"nW2Bi{t2c}{f2c}"][:],
                                                 rhs=bi[:], start=False, stop=(t2c == 1))
                                nc.tensor.matmul(pzi[:], lhsT=cs[f"W2Bi{t2c}{f2c}"][:],
                                                 rhs=br[:], start=(t2c == 0), stop=False)
                                nc.tensor.matmul(pzi[:], lhsT=cs[f"W2Br{t2c}{f2c}"][:],
                                                 rhs=bi[:], start=False, stop=(t2c == 1))
                            nc.scalar.copy(zr[:], pzr[:])
                            nc.scalar.copy(zi[:], pzi[:])
                            Z2[(inp, f2c)] = (zr, zi)

                    S2 = {}
                    tmpc = dp.tile([128, rb2 * 128], F32, tag="tmpc", name="tmpc")
                    for f2c in range(2):
                        zcr, zci = Z2[("c", f2c)]
                        ztr, zti = Z2[("t", f2c)]
                        sr = dp.tile([128, rb2 * 128], cdt, tag=f"S2r{f2c}", name=f"S2r{f2c}")
                        si = dp.tile([128, rb2 * 128], cdt, tag=f"S2i{f2c}", name=f"S2i{f2c}")
                        nc.vector.tensor_tensor(sr[:], zcr[:], ztr[:], op=AT.mult)
                        nc.vector.tensor_tensor(tmpc[:], zci[:], zti[:], op=AT.mult)
                        nc.vector.tensor_tensor(sr[:], sr[:], tmpc[:], op=AT.add)
                        nc.vector.tensor_tensor(si[:], zci[:], ztr[:], op=AT.mult)
                        nc.vector.tensor_tensor(tmpc[:], zcr[:], zti[:], op=AT.mult)
                        nc.vector.tensor_tensor(si[:], si[:], tmpc[:], op=AT.subtract)
                        S2[f2c] = (sr, si)

                    H2r = dp.tile([128, rb2 * 256], cdt, tag="H2r", name="H2r")
                    H2i = dp.tile([128, rb2 * 256], cdt, tag="H2i", name="H2i")
                    for g in range(rb2 // 2):
                        pgr = pp.tile([128, 512], F32, tag="inv", name="inv")
                        pgi = pp.tile([128, 512], F32, tag="inv", name="inv")
                        for q in range(2):
                            qq = g * 2 + q
                            sl, osl = bass.ts(qq, 128), bass.ts(q, 256)
                            for f2c in range(2):
                                sr, si = S2[f2c]
                                nc.tensor.matmul(pgr[:, osl], lhsT=sr[:, sl],
                                                 rhs=cs[f"V2Br{f2c}"][:],
                                                 start=(f2c == 0), stop=False)
                                nc.tensor.matmul(pgr[:, osl], lhsT=si[:, sl],
                                                 rhs=cs[f"nV2Bi{f2c}"][:],
                                                 start=False, stop=(f2c == 1))
                                nc.tensor.matmul(pgi[:, osl], lhsT=sr[:, sl],
                                                 rhs=cs[f"V2Bi{f2c}"][:],
                                                 start=(f2c == 0), stop=False)
                                nc.tensor.matmul(pgi[:, osl], lhsT=si[:, sl],
                                                 rhs=cs[f"V2Br{f2c}"][:],
                                                 start=False, stop=(f2c == 1))
                        gsl = bass.ts(g, 512)
                        _cmul_psum(nc, dp, "tw", H2r[:, gsl], H2i[:, gsl],
                                   pgr[:], pgi[:],
                                   _b3(cs["itwBr"][:], 2, 256),
                                   _b3(cs["itwBi"][:], 2, 256), 256, 2)

                    for g in range(rb2 // 2):
                        gsl = bass.ts(g, 512)
                        pcc = pp.tile([128, 512], F32, tag="cc", name="cc")
                        nc.tensor.matmul(pcc[:], lhsT=cs["V1Br"][:],
                                         rhs=H2r[:, gsl], start=True, stop=False)
                        nc.tensor.matmul(pcc[:], lhsT=cs["nV1Bi"][:],
                                         rhs=H2i[:, gsl], start=False, stop=True)
                        csl = slice((r0 + g * 2) * 256, (r0 + g * 2 + 2) * 256)
                        ccv = ccm_all[:, csl].rearrange("p (a b) -> p a b", b=256)
                        nc.vector.scalar_tensor_tensor(
                            ccv, pcc[:].rearrange("p (a b) -> p a b", b=256),
                            1.0, _b3(cs["maskB"][:], 2, 256),
                            op0=AT.bypass, op1=AT.add)
                        nc.vector.tensor_reduce(
                            allmax[:, r0 + g * 2:r0 + g * 2 + 2], ccv,
                            axis=AX.X, op=AT.max)

            # ---------------- D) argmax -> shifts ----------------
            with tc.tile_pool(name="amax", bufs=1) as dp:
                pt = pp.tile([rows, 128], F32, tag="st1", name="st1")
                nc.tensor.transpose(pt[:], allmax[:, 0:rows], cs["ident"][:])
                tmax = dp.tile([rows, 128], F32, tag="tmax", name="tmax")
                nc.scalar.copy(tmax[:], pt[:])
                rowmax = dp.tile([rows, 1], F32, tag="rowmax", name="rowmax")
                nc.vector.tensor_reduce(rowmax[:], tmax[:], axis=AX.X, op=AT.max)
                prm = pp.tile([1, rows], F32, tag="st2", name="st2")
                nc.tensor.transpose(prm[:], rowmax[:], cs["ident"][0:rows, 0:rows])
                rmT = dp.tile([1, rows], F32, tag="rmT", name="rmT")
                nc.scalar.copy(rmT[:], prm[:])
                pmb = pp.tile([128, rows], F32, tag="inv", name="inv")
                nc.tensor.matmul(pmb[:], lhsT=cs["ones1x128"][:], rhs=rmT[:],
                                 start=True, stop=True)
                Mb = dp.tile([128, rows], F32, tag="Mb", name="Mb")
                nc.scalar.copy(Mb[:], pmb[:])

                eqm = dp.tile([128, min(rows, 8) * 256], BF16, tag="eqm", name="eqm")
                selm = dp.tile([128, min(rows, 8) * 256], F32, tag="selm", name="selm")
                for bb in range(max(1, rows // 8)):
                    csl = bass.ts(bb, min(rows, 8) * 256)
                    nr8 = min(rows, 8)
                    mbb = Mb[:, bb * nr8:(bb + 1) * nr8]\
                        .rearrange("p (a b) -> p a b", b=1)\
                        .to_broadcast([128, nr8, 256])
                    ccv = ccm_all[:, csl].rearrange("p (a b) -> p a b", b=256)
                    nc.vector.tensor_tensor(
                        eqm[:].rearrange("p (a b) -> p a b", b=256),
                        ccv, mbb, op=AT.is_equal)
                    nc.vector.tensor_tensor(
                        selm[:].rearrange("p (a b) -> p a b", b=256),
                        eqm[:].rearrange("p (a b) -> p a b", b=256),
                        _b3(cs["shvB"][:], nr8, 256), op=AT.mult)
                    nc.vector.tensor_reduce(
                        allmin[:, bb * nr8:(bb + 1) * nr8],
                        selm[:].rearrange("p (a b) -> p a b", b=256),
                        axis=AX.X, op=AT.min)
                pt2 = pp.tile([rows, 128], F32, tag="cc", name="cc")
                nc.tensor.transpose(pt2[:], allmin[:, 0:rows], cs["ident"][:])
                tmin = dp.tile([rows, 128], F32, tag="tmin", name="tmin")
                nc.scalar.copy(tmin[:], pt2[:])
                nc.vector.tensor_reduce(shifts[:], tmin[:], axis=AX.X, op=AT.min)
                nc.vector.tensor_scalar_add(shifts[:], shifts[:], BIGL + float(START0))

                # start = (7040 + shift) mod 14337
                m1 = dp.tile([rows, 1], F32, tag="m1", name="m1")
                nc.vector.tensor_scalar(out=m1[:], in0=shifts[:], scalar1=0.0,
                                        scalar2=None, op0=AT.is_lt)
                nc.vector.scalar_tensor_tensor(
                    shifts[:], m1[:], float(CONV_LEN), shifts[:],
                    op0=AT.mult, op1=AT.add)
                nc.vector.tensor_scalar(out=m1[:], in0=shifts[:],
                                        scalar1=float(CONV_LEN), scalar2=None,
                                        op0=AT.is_ge)
                nc.vector.scalar_tensor_tensor(
                    shifts[:], m1[:], float(-CONV_LEN), shifts[:],
                    op0=AT.mult, op1=AT.add)

                idxf = dp.tile([rows, CROP], F32, tag="idxf", name="idxf")
                nc.vector.tensor_tensor(idxf[:], cs["winidx"][0:rows, :],
                                        shifts[:].to_broadcast([rows, CROP]),
                                        op=AT.add)
                idxi = dp.tile([rows, CROP], I32, tag="idxi", name="idxi")
                nc.vector.tensor_copy(idxi[:], idxf[:])
                w = dp.tile([rows, CROP], F32, tag="wg", name="wg")
                nc.gpsimd.indirect_dma_start(
                    out=w[:], out_offset=None,
                    in_=scratch.ap().rearrange("r p -> (r p)").rearrange(
                        "(a b) -> a b", b=1),
                    in_offset=bass.IndirectOffsetOnAxis(ap=idxi[:], axis=0),
                )
                tw_ = dp.tile([rows, CROP], F32, tag="twin", name="twin")
                nc.sync.dma_start(tw_[:], target[:, START0:START0 + CROP])
                nc.vector.tensor_tensor(w[:], w[:], tw_[:], op=AT.subtract)
                convacc = dp.tile([rows, 1], F32, tag="convacc", name="convacc")
                nc.vector.scalar_tensor_tensor(
                    tw_[:], w[:], 1.0, w[:], op0=AT.bypass, op1=AT.mult,
                    accum_out=convacc[:])

                a0 = dp.tile([128, 1], F32, tag="a0", name="a0")
                nc.vector.tensor_reduce(a0[:], astf_acc[:], axis=AX.X, op=AT.add)
                psa = pp.tile([1, 1], F32, tag="st1", name="st1")
                nc.tensor.matmul(psa[:], lhsT=a0[:], rhs=cs["ones128"][:],
                                 start=True, stop=True)
                psc = pp.tile([1, 1], F32, tag="st2", name="st2")
                nc.tensor.matmul(psc[:], lhsT=convacc[:], rhs=cs["ones64"][0:rows, :],
                                 start=True, stop=True)
                nc.scalar.copy(outt[:, 0:1], psa[:])
                nc.scalar.copy(outt[:, 1:2], psc[:])
                nc.sync.dma_start(out[:], outt[:])

    nc.finalize()
    return nc, consts


_CACHE = {}


def get_built(cdt=F32):
    key = str(cdt)
    if key not in _CACHE:
        _CACHE[key] = build_nc(cdt=cdt)
    return _CACHE[key]


LAST_RESULT = {}


def kernel(pred_astf, true_astf, egf, target_waveform):
    import os
    from concourse.bass_utils import run_bass_kernel_spmd
    cdt = BF16 if os.environ.get("CONVALIGN_BF16") == "1" else F32
    nc, consts = get_built(cdt)
    if cdt != F32:
        import ml_dtypes
        from kernel import make_consts as _mk  # noqa
        mmnames = _mm_const_names()
        consts = {k: (v.astype(ml_dtypes.bfloat16) if k in mmnames else v)
                  for k, v in consts.items()}
    pred_astf = np.ascontiguousarray(np.asarray(pred_astf, np.float32))
    true_astf = np.ascontiguousarray(np.asarray(true_astf, np.float32))
    egf = np.ascontiguousarray(np.asarray(egf, np.float32))
    target_waveform = np.ascontiguousarray(
        np.asarray(target_waveform, np.float32))
    B = pred_astf.shape[0]
    per = B // NCORES
    in_maps = []
    for i in range(NCORES):
        sl = slice(i * per, (i + 1) * per)
        m = {"pred": pred_astf[sl], "true": true_astf[sl],
             "egf": egf[sl], "target": target_waveform[sl]}
        m.update(consts)
        in_maps.append(m)
    import os
    trace = os.environ.get("CONVALIGN_TRACE") == "1"
    res = run_bass_kernel_spmd(nc, in_maps, core_ids=list(range(NCORES)),
                               trace=trace)
    LAST_RESULT["res"] = res
    sums = np.stack([res.results[i]["out"][0] for i in range(NCORES)])
    loss_astf = np.float32(sums[:, 0].sum() / (B * L1))
    loss_conv = np.float32(sums[:, 1].sum() / (B * CROP))
    total = np.float32(loss_astf + loss_conv)
    return total, loss_astf, loss_conv



# revision 2
# speedup vs baseline: 1.3314x; 1.3314x over previous
"""Trainium2 Bass kernel for nn_ConvAlignLoss (8-core data parallel).

Self-contained: hardcodes shapes; imports concourse from /opt/trn_rl_repo.

Per core (R=64 rows):
  loss_astf partial: sum((pred-true)^2)
  conv = irfft16384(fft(pred) * conj(fft(egf_pad)))[:14337]  (2-stage matmul FFT)
  cc   = irfft32768(fft(conv_pad) * conj(fft(target_pad)))
  shift = mapped masked argmax of cc (== reference argmax over n=28673)
  loss_conv partial: sum((conv[(7040+i+shift) % 14337] - target[7040+i])^2)
Host combines the 8 cores' (sum_astf, sum_conv) into the scalar losses.

FFT structure (N = 128*N2):
  FWD:  D[t1,t2]=x[N2*t1+t2]; A[t2,f1]=sum_t1 D*W1 (data-stationary matmul);
        B=A*tw; Z[f2,f1]=sum_t2 W2[t2,f2]*B.   Z2d[f2,f1] == X[f1+128*f2]
  INV:  G[f1,t2]=sum_f2 S2d[f2,f1]*V2[f2,t2] (S stationary); H=G*itw;
        x2d[t1,t2]=(1/N) Re(sum_f1 V1[f1,t1]*H[f1,t2])
"""
import sys

sys.path.insert(0, "/opt/trn_rl_repo")

import numpy as np
import concourse.bass as bass
import concourse.bacc as bacc
import concourse.mybir as mybir
from concourse import tile

F32 = mybir.dt.float32
BF16 = mybir.dt.bfloat16
I32 = mybir.dt.int32
AT = mybir.AluOpType
AX = mybir.AxisListType

R = 64
NCORES = 8
L1, L2 = 16384, 2048
CONV_LEN = L1 - L2 + 1      # 14337
N_A, N_B = 16384, 32768
GAP_LO, GAP_HI = CONV_LEN, N_B - CONV_LEN + 1   # gap [14337, 18432)
CROP = 256
START0 = (CONV_LEN - CROP) // 2                 # 7040
PITCH = 14720
BIGL = float(2 ** 23)


def _dft(n, sign):
    k = np.arange(n)
    return np.exp(sign * 2j * np.pi * np.outer(k, k) / n)


def make_consts():
    c = {}

    def put(name, arr, dt=np.float32):
        c[name] = np.ascontiguousarray(np.asarray(arr, np.float64)).astype(dt)

    W1 = _dft(128, -1)
    put("W1r", W1.real); put("W1i", W1.imag); put("nW1i", -W1.imag)
    twA = np.exp(-2j * np.pi * np.outer(np.arange(128), np.arange(128)) / N_A)
    put("twAr", twA.real); put("twAi", twA.imag)
    V2A = _dft(128, +1)
    put("V2Ar", V2A.real); put("V2Ai", V2A.imag); put("nV2Ai", -V2A.imag)
    itwA = np.exp(2j * np.pi * np.outer(np.arange(128), np.arange(128)) / N_A)
    put("itwAr", itwA.real); put("itwAi", itwA.imag)
    V1A = _dft(128, +1) / N_A
    put("V1Ar", V1A.real); put("nV1Ai", -V1A.imag)

    W2B = _dft(256, -1)          # [t2, f2]
    for a in range(2):
        for b in range(2):
            blk = W2B[a * 128:(a + 1) * 128, b * 128:(b + 1) * 128]
            put(f"W2Br{a}{b}", blk.real)
            put(f"W2Bi{a}{b}", blk.imag)
            put(f"nW2Bi{a}{b}", -blk.imag)
    twB = np.exp(-2j * np.pi * np.outer(np.arange(256), np.arange(128)) / N_B)
    for a in range(2):
        put(f"twBr{a}", twB.real[a * 128:(a + 1) * 128])
        put(f"twBi{a}", twB.imag[a * 128:(a + 1) * 128])
    V2B = _dft(256, +1)          # [f2, t2]
    for a in range(2):
        blk = V2B[a * 128:(a + 1) * 128, :]
        put(f"V2Br{a}", blk.real)
        put(f"V2Bi{a}", blk.imag)
        put(f"nV2Bi{a}", -blk.imag)
    itwB = np.exp(2j * np.pi * np.outer(np.arange(128), np.arange(256)) / N_B)
    put("itwBr", itwB.real); put("itwBi", itwB.imag)
    V1B = _dft(128, +1) / N_B
    put("V1Br", V1B.real); put("nV1Bi", -V1B.imag)

    put("ident", np.eye(128))
    put("ones1x128", np.ones((1, 128)))
    put("ones128", np.ones((128, 1)))
    put("ones64", np.ones((64, 1)))

    j = np.arange(128)[:, None] * 256 + np.arange(256)[None, :]   # [t1, t2]
    gap = (j >= GAP_LO) & (j < GAP_HI)
    put("maskB", np.where(gap, -1e30, 0.0))
    shiftval = np.where(j <= CONV_LEN - 1, j - (CONV_LEN - 1), j - GAP_HI + 1)
    put("shvB", np.where(gap, 0.0, shiftval - BIGL))
    put("winidx", np.arange(R)[:, None] * PITCH
        + np.arange(CROP)[None, :])                               # [64, 256]
    return c


def _b3(ap, n, inner):
    """[128, inner] const AP -> [128, n, inner] broadcast over middle dim."""
    return ap.rearrange("p (a b) -> p a b", a=1).to_broadcast([128, n, inner])


def _cmul_psum(nc, pool, tag, outr, outi, pr, pi, twr, twi, inner, n):
    """(outr + i outi) = (pr + i pi) * (twr + i twi); p* in PSUM, tw const APs
    broadcast over n blocks of `inner`. outr/outi are SBUF APs [128, n*inner]."""
    tmp = pool.tile([128, n * inner], F32, tag="twtmp", name="twtmp")
    orv = outr.rearrange("p (a b) -> p a b", b=inner)
    oiv = outi.rearrange("p (a b) -> p a b", b=inner)
    prv = pr.rearrange("p (a b) -> p a b", b=inner)
    piv = pi.rearrange("p (a b) -> p a b", b=inner)
    tv = tmp[:].rearrange("p (a b) -> p a b", b=inner)
    nc.vector.tensor_tensor(orv, prv, twr, op=AT.mult)
    nc.vector.tensor_tensor(tv, piv, twi, op=AT.mult)
    nc.vector.tensor_tensor(orv, orv, tv, op=AT.subtract)
    nc.vector.tensor_tensor(oiv, prv, twi, op=AT.mult)
    nc.vector.tensor_tensor(tv, piv, twr, op=AT.mult)
    nc.vector.tensor_tensor(oiv, oiv, tv, op=AT.add)


def _mm_const_names():
    s = {"W1r", "W1i", "nW1i", "V2Ar", "V2Ai", "nV2Ai", "V1Ar", "nV1Ai",
         "itwAr", "itwAi", "twAr", "twAi", "itwBr", "itwBi", "V1Br",
         "nV1Bi"}
    s |= {f"W2Br{a}{b}" for a in range(2) for b in range(2)}
    s |= {f"W2Bi{a}{b}" for a in range(2) for b in range(2)}
    s |= {f"nW2Bi{a}{b}" for a in range(2) for b in range(2)}
    s |= {f"twBr{a}" for a in range(2)} | {f"twBi{a}" for a in range(2)}
    s |= {f"V2Br{a}" for a in range(2)} | {f"V2Bi{a}" for a in range(2)}
    s |= {f"nV2Bi{a}" for a in range(2)}
    return s


def build_nc(cdt=F32, rows=R, rbb=8, rb2=4):
    nc = bacc.Bacc("TRN2", target_bir_lowering=False, debug=False,
                   num_devices=NCORES)
    consts = make_consts()

    pred = nc.dram_tensor("pred", [rows, L1], F32, kind="ExternalInput")
    true_ = nc.dram_tensor("true", [rows, L1], F32, kind="ExternalInput")
    egf = nc.dram_tensor("egf", [rows, L2], F32, kind="ExternalInput")
    target = nc.dram_tensor("target", [rows, CONV_LEN], F32,
                            kind="ExternalInput")
    out = nc.dram_tensor("out", [1, 2], F32, kind="ExternalOutput")
    scratch = nc.dram_tensor("scratch", [rows, PITCH], F32)

    MM_CONST = _mm_const_names()

    cdram = {}
    for name, arr in consts.items():
        cdt_n = cdt if name in MM_CONST else F32
        cdram[name] = nc.dram_tensor(name, list(arr.shape), cdt_n,
                                     kind="ExternalInput")

    nb1, nb2 = rows // rbb, rows // rb2

    with tile.TileContext(nc) as tc:
        with (
            tc.tile_pool(name="consts", bufs=1) as cpool,
            tc.tile_pool(name="keep", bufs=1) as kpool,
            tc.tile_pool(name="ps", bufs=2, space="PSUM") as pp,
        ):
            cs = {}
            for name, arr in consts.items():
                dt = cdt if name in MM_CONST else F32
                t = cpool.tile(list(arr.shape), dt, tag=f"c_{name}", name=f"c_{name}")
                nc.sync.dma_start(t[:], cdram[name][:])
                cs[name] = t

            allmax = kpool.tile([128, rows], F32, tag="allmax", name="allmax")
            allmin = kpool.tile([128, rows], F32, tag="allmin", name="allmin")
            ccm_all = kpool.tile([128, rows * 256], BF16, tag="ccm", name="ccm")
            astf_acc = kpool.tile([128, 8], F32, tag="astfacc", name="astfacc")
            shifts = kpool.tile([rows, 1], F32, tag="shifts", name="shifts")
            outt = kpool.tile([1, 2], F32, tag="outt", name="outt")

            # ---------------- A) astf ----------------
            predf = pred.ap().rearrange("r l -> (r l)").rearrange(
                "(p f) -> p f", p=128)
            truef = true_.ap().rearrange("r l -> (r l)").rearrange(
                "(p f) -> p f", p=128)
            fch = rows * L1 // 128 // 8
            with tc.tile_pool(name="astf", bufs=2) as apool:
                for i in range(8):
                    tp = apool.tile([128, fch], F32, tag="ap", name="ap")
                    tt = apool.tile([128, fch], F32, tag="at", name="at")
                    sl = bass.ts(i, fch)
                    nc.sync.dma_start(tp[:], predf[:, sl])
                    nc.sync.dma_start(tt[:], truef[:, sl])
                    nc.vector.tensor_tensor(tt[:], tp[:], tt[:], op=AT.subtract)
                    nc.vector.scalar_tensor_tensor(
                        tp[:], tt[:], 1.0, tt[:], op0=AT.bypass, op1=AT.mult,
                        accum_out=astf_acc[:, i:i + 1])

            # ---------------- B) 16K level ----------------
            with tc.tile_pool(name="p16", bufs=1) as dp:
                for b in range(nb1):
                    r0 = b * rbb
                    Dp = dp.tile([128, rbb * 128], cdt, tag="Dp", name="Dp")
                    De = dp.tile([128, rbb * 128], cdt, tag="De", name="De")
                    nc.scalar.memzero(De[:])
                    if cdt == F32:
                        for q in range(rbb):
                            r = r0 + q
                            nc.sync.dma_start(
                                Dp[:, bass.ts(q, 128)],
                                pred[r, :].rearrange("(a b) -> a b", a=128))
                            nc.sync.dma_start(
                                De[:16, bass.ts(q, 128)],
                                egf[r, :].rearrange("(a b) -> a b", a=16))
                    else:
                        Dst = dp.tile([128, rbb * 128], F32, tag="Dst", name="Dst")
                        Est = dp.tile([16, rbb * 128], F32, tag="Est", name="Est")
                        for q in range(rbb):
                            r = r0 + q
                            nc.sync.dma_start(
                                Dst[:, bass.ts(q, 128)],
                                pred[r, :].rearrange("(a b) -> a b", a=128))
                            nc.sync.dma_start(
                                Est[:16, bass.ts(q, 128)],
                                egf[r, :].rearrange("(a b) -> a b", a=16))
                        nc.scalar.copy(Dp[:], Dst[:])
                        nc.scalar.copy(De[:16, :], Est[:16, :])

                    Bs = {k: dp.tile([128, rbb * 128], cdt, tag=f"B{k}", name=f"B{k}")
                          for k in ("pr", "pi", "er", "ei")}
                    for g in range(rbb // 4):
                        gsl = bass.ts(g, 512)
                        for inp, D in (("p", Dp), ("e", De)):
                            pa = pp.tile([128, 512], F32, tag="st1", name="st1")
                            pai = pp.tile([128, 512], F32, tag="st1", name="st1")
                            for q in range(4):
                                qq = g * 4 + q
                                sl, osl = bass.ts(qq, 128), bass.ts(q, 128)
                                nc.tensor.matmul(pa[:, osl], lhsT=D[:, sl],
                                                 rhs=cs["W1r"][:],
                                                 start=True, stop=True)
                                nc.tensor.matmul(pai[:, osl], lhsT=D[:, sl],
                                                 rhs=cs["W1i"][:],
                                                 start=True, stop=True)
                            _cmul_psum(nc, dp, "tw",
                                       Bs[inp + "r"][:, gsl], Bs[inp + "i"][:, gsl],
                                       pa[:], pai[:],
                                       _b3(cs["twAr"][:], 4, 128),
                                       _b3(cs["twAi"][:], 4, 128), 128, 4)

                    Zs = {k: dp.tile([128, rbb * 128], cdt, tag=f"Z{k}", name=f"Z{k}")
                          for k in ("pr", "pi", "er", "ei")}
                    for g in range(rbb // 4):
                        gsl = bass.ts(g, 512)
                        for inp in ("p", "e"):
                            br, bi = Bs[inp + "r"], Bs[inp + "i"]
                            pzr = pp.tile([128, 512], F32, tag="st2", name="st2")
                            pzi = pp.tile([128, 512], F32, tag="st2", name="st2")
                            nc.tensor.matmul(pzr[:], lhsT=cs["W1r"][:],
                                             rhs=br[:, gsl], start=True, stop=False)
                            nc.tensor.matmul(pzr[:], lhsT=cs["nW1i"][:],
                                             rhs=bi[:, gsl], start=False, stop=True)
                            nc.tensor.matmul(pzi[:], lhsT=cs["W1i"][:],
                                             rhs=br[:, gsl], start=True, stop=False)
                            nc.tensor.matmul(pzi[:], lhsT=cs["W1r"][:],
                                             rhs=bi[:, gsl], start=False, stop=True)
                            nc.scalar.copy(Zs[inp + "r"][:, gsl], pzr[:])
                            nc.scalar.copy(Zs[inp + "i"][:, gsl], pzi[:])

                    Sr = dp.tile([128, rbb * 128], cdt, tag="Sr", name="Sr")
                    Si = dp.tile([128, rbb * 128], cdt, tag="Si", name="Si")
                    tmpb = dp.tile([128, rbb * 128], F32, tag="tmpbig", name="tmpbig")
                    nc.vector.tensor_tensor(Sr[:], Zs["pr"][:], Zs["er"][:], op=AT.mult)
                    nc.vector.tensor_tensor(tmpb[:], Zs["pi"][:], Zs["ei"][:], op=AT.mult)
                    nc.vector.tensor_tensor(Sr[:], Sr[:], tmpb[:], op=AT.add)
                    nc.vector.tensor_tensor(Si[:], Zs["pi"][:], Zs["er"][:], op=AT.mult)
                    nc.vector.tensor_tensor(tmpb[:], Zs["pr"][:], Zs["ei"][:], op=AT.mult)
                    nc.vector.tensor_tensor(Si[:], Si[:], tmpb[:], op=AT.subtract)

                    Hr = dp.tile([128, rbb * 128], cdt, tag="Hr", name="Hr")
                    Hi = dp.tile([128, rbb * 128], cdt, tag="Hi", name="Hi")
                    for g in range(rbb // 4):
                        gsl = bass.ts(g, 512)
                        pgr = pp.tile([128, 512], F32, tag="inv", name="inv")
                        pgi = pp.tile([128, 512], F32, tag="inv", name="inv")
                        for q in range(4):
                            qq = g * 4 + q
                            sl, osl = bass.ts(qq, 128), bass.ts(q, 128)
                            nc.tensor.matmul(pgr[:, osl], lhsT=Sr[:, sl],
                                             rhs=cs["V2Ar"][:], start=True, stop=False)
                            nc.tensor.matmul(pgr[:, osl], lhsT=Si[:, sl],
                                             rhs=cs["nV2Ai"][:], start=False, stop=True)
                            nc.tensor.matmul(pgi[:, osl], lhsT=Sr[:, sl],
                                             rhs=cs["V2Ai"][:], start=True, stop=False)
                            nc.tensor.matmul(pgi[:, osl], lhsT=Si[:, sl],
                                             rhs=cs["V2Ar"][:], start=False, stop=True)
                        _cmul_psum(nc, dp, "tw", Hr[:, gsl], Hi[:, gsl],
                                   pgr[:], pgi[:],
                                   _b3(cs["itwAr"][:], 4, 128),
                                   _b3(cs["itwAi"][:], 4, 128), 128, 4)

                    convSB = dp.tile([128, rbb * 128], F32, tag="convSB", name="convSB")
                    for g in range(rbb // 4):
                        gsl = bass.ts(g, 512)
                        pc = pp.tile([128, 512], F32, tag="cc", name="cc")
                        nc.tensor.matmul(pc[:], lhsT=cs["V1Ar"][:],
                                         rhs=Hr[:, gsl], start=True, stop=False)
                        nc.tensor.matmul(pc[:], lhsT=cs["nV1Ai"][:],
                                         rhs=Hi[:, gsl], start=False, stop=True)
                        nc.scalar.copy(convSB[:, gsl], pc[:])

                    for q in range(rbb):
                        r = r0 + q
                        csl = bass.ts(q, 128)
                        nc.sync.dma_start(
                            scratch[r, 0:14336].rearrange("(a b) -> a b", a=112),
                            convSB[0:112, csl])
                        nc.sync.dma_start(
                            scratch[r, 14336:14337].rearrange("(a b) -> a b", a=1),
                            convSB[112:113, q * 128:q * 128 + 1])
                        nc.sync.dma_start(
                            scratch[r, 14337:14593].rearrange("(a b) -> a b", a=2),
                            convSB[0:2, csl])

            # ---------------- C) 32K level ----------------
            with tc.tile_pool(name="p32", bufs=1) as dp:
                for b in range(nb2):
                    r0 = b * rb2
                    D2c = dp.tile([128, rb2 * 256], cdt, tag="D2c", name="D2c")
                    D2t = dp.tile([128, rb2 * 256], cdt, tag="D2t", name="D2t")
                    nc.scalar.memzero(D2c[:])
                    nc.scalar.memzero(D2t[:])
                    if cdt == F32:
                        tgc, tgt_ = D2c, D2t
                    else:
                        tgc = dp.tile([128, rb2 * 256], F32, tag="D2cs", name="D2cs")
                        tgt_ = dp.tile([128, rb2 * 256], F32, tag="D2ts", name="D2ts")
                        nc.scalar.memzero(tgc[:])
                        nc.scalar.memzero(tgt_[:])
                    for q in range(rb2):
                        r = r0 + q
                        sl = bass.ts(q, 256)
                        nc.sync.dma_start(
                            tgc[0:56, sl],
                            scratch[r, 0:14336].rearrange("(a b) -> a b", a=56))
                        nc.sync.dma_start(
                            tgc[56:57, q * 256:q * 256 + 1],
                            scratch[r, 14336:14337].rearrange("(a b) -> a b", a=1))
                        nc.sync.dma_start(
                            tgt_[0:56, sl],
                            target[r, 0:14336].rearrange("(a b) -> a b", a=56))
                        nc.sync.dma_start(
                            tgt_[56:57, q * 256:q * 256 + 1],
                            target[r, 14336:14337].rearrange("(a b) -> a b", a=1))
                    if cdt != F32:
                        nc.scalar.copy(D2c[0:57, :], tgc[0:57, :])
                        nc.scalar.copy(D2t[0:57, :], tgt_[0:57, :])

                    B2 = {}
                    for c in range(2):
                        for inp, D in (("c", D2c), ("t", D2t)):
                            br = dp.tile([128, rb2 * 128], cdt, tag=f"B2r{c}{inp}", name=f"B2r{c}{inp}")
                            bi = dp.tile([128, rb2 * 128], cdt, tag=f"B2i{c}{inp}", name=f"B2i{c}{inp}")
                            pa = pp.tile([128, rb2 * 128], F32, tag="st1", name="st1")
                            pai = pp.tile([128, rb2 * 128], F32, tag="st1", name="st1")
                            for q in range(rb2):
                                dsl = slice(q * 256 + c * 128,
                                            q * 256 + c * 128 + 128)
                                osl = bass.ts(q, 128)
                                nc.tensor.matmul(pa[:, osl], lhsT=D[:, dsl],
                                                 rhs=cs["W1r"][:],
                                                 start=True, stop=True)
                                nc.tensor.matmul(pai[:, osl], lhsT=D[:, dsl],
                                                 rhs=cs["W1i"][:],
                                                 start=True, stop=True)
                            _cmul_psum(nc, dp, "tw", br[:], bi[:], pa[:], pai[:],
                                       _b3(cs[f"twBr{c}"][:], rb2, 128),
                                       _b3(cs[f"twBi{c}"][:], rb2, 128), 128, rb2)
                            B2[(c, inp)] = (br, bi)

                    Z2 = {}
                    for inp in ("c", "t"):
                        for f2c in range(2):
                            zr = dp.tile([128, rb2 * 128], cdt, tag=f"Z2r{inp}{f2c}", name=f"Z2r{inp}{f2c}")
                            zi = dp.tile([128, rb2 * 128], cdt, tag=f"Z2i{inp}{f2c}", name=f"Z2i{inp}{f2c}")
                            pzr = pp.tile([128, rb2 * 128], F32, tag="st2", name="st2")
                            pzi = pp.tile([128, rb2 * 128], F32, tag="st2", name="st2")
                            for t2c in range(2):
                                br, bi = B2[(t2c, inp)]
                                nc.tensor.matmul(pzr[:], lhsT=cs[f"W2Br{t2c}{f2c}"][:],
                                                 rhs=br[:], start=(t2c == 0), stop=False)
                                nc.tensor.matmul(pzr[:], lhsT=cs[f"nW2Bi{t2c}{f2c}"][:],
                                                 rhs=bi[:], start=False, stop=(t2c == 1))
                                nc.tensor.matmul(pzi[:], lhsT=cs[f"W2Bi{t2c}{f2c}"][:],
                                                 rhs=br[:], start=(t2c == 0), stop=False)
                                nc.tensor.matmul(pzi[:], lhsT=cs[f"W2Br{t2c}{f2c}"][:],
                                                 rhs=bi[:], start=False, stop=(t2c == 1))
                            nc.scalar.copy(zr[:], pzr[:])
                            nc.scalar.copy(zi[:], pzi[:])
                            Z2[(inp, f2c)] = (zr, zi)

                    S2 = {}
                    tmpc = dp.tile([128, rb2 * 128], F32, tag="tmpc", name="tmpc")
                    for f2c in range(2):
                        zcr, zci = Z2[("c", f2c)]
                        ztr, zti = Z2[("t", f2c)]
                        sr = dp.tile([128, rb2 * 128], cdt, tag=f"S2r{f2c}", name=f"S2r{f2c}")
                        si = dp.tile([128, rb2 * 128], cdt, tag=f"S2i{f2c}", name=f"S2i{f2c}")
                        nc.vector.tensor_tensor(sr[:], zcr[:], ztr[:], op=AT.mult)
                        nc.vector.tensor_tensor(tmpc[:], zci[:], zti[:], op=AT.mult)
                        nc.vector.tensor_tensor(sr[:], sr[:], tmpc[:], op=AT.add)
                        nc.vector.tensor_tensor(si[:], zci[:], ztr[:], op=AT.mult)
                        nc.vector.tensor_tensor(tmpc[:], zcr[:], zti[:], op=AT.mult)
                        nc.vector.tensor_tensor(si[:], si[:], tmpc[:], op=AT.subtract)
                        S2[f2c] = (sr, si)

                    H2r = dp.tile([128, rb2 * 256], cdt, tag="H2r", name="H2r")
                    H2i = dp.tile([128, rb2 * 256], cdt, tag="H2i", name="H2i")
                    for g in range(rb2 // 2):
                        pgr = pp.tile([128, 512], F32, tag="inv", name="inv")
                        pgi = pp.tile([128, 512], F32, tag="inv", name="inv")
                        for q in range(2):
                            qq = g * 2 + q
                            sl, osl = bass.ts(qq, 128), bass.ts(q, 256)
                            for f2c in range(2):
                                sr, si = S2[f2c]
                                nc.tensor.matmul(pgr[:, osl], lhsT=sr[:, sl],
                                                 rhs=cs[f"V2Br{f2c}"][:],
                                                 start=(f2c == 0), stop=False)
                                nc.tensor.matmul(pgr[:, osl], lhsT=si[:, sl],
                                                 rhs=cs[f"nV2Bi{f2c}"][:],
                                                 start=False, stop=(f2c == 1))
                                nc.tensor.matmul(pgi[:, osl], lhsT=sr[:, sl],
                                                 rhs=cs[f"V2Bi{f2c}"][:],
                                                 start=(f2c == 0), stop=False)
                                nc.tensor.matmul(pgi[:, osl], lhsT=si[:, sl],
                                                 rhs=cs[f"V2Br{f2c}"][:],
                                                 start=False, stop=(f2c == 1))
                        gsl = bass.ts(g, 512)
                        _cmul_psum(nc, dp, "tw", H2r[:, gsl], H2i[:, gsl],
                                   pgr[:], pgi[:],
                                   _b3(cs["itwBr"][:], 2, 256),
                                   _b3(cs["itwBi"][:], 2, 256), 256, 2)

                    for g in range(rb2 // 2):
                        gsl = bass.ts(g, 512)
                        pcc = pp.tile([128, 512], F32, tag="cc", name="cc")
                        nc.tensor.matmul(pcc[:], lhsT=cs["V1Br"][:],
                                         rhs=H2r[:, gsl], start=True, stop=False)
                        nc.tensor.matmul(pcc[:], lhsT=cs["nV1Bi"][:],
                                         rhs=H2i[:, gsl], start=False, stop=True)
                        csl = slice((r0 + g * 2) * 256, (r0 + g * 2 + 2) * 256)
                        ccv = ccm_all[:, csl].rearrange("p (a b) -> p a b", b=256)
                        nc.vector.scalar_tensor_tensor(
                            ccv, pcc[:].rearrange("p (a b) -> p a b", b=256),
                            1.0, _b3(cs["maskB"][:], 2, 256),
                            op0=AT.bypass, op1=AT.add)
                        nc.vector.tensor_reduce(
                            allmax[:, r0 + g * 2:r0 + g * 2 + 2], ccv,
                            axis=AX.X, op=AT.max)

            # ---------------- D) argmax -> shifts ----------------
            with tc.tile_pool(name="amax", bufs=1) as dp:
                pt = pp.tile([rows, 128], F32, tag="st1", name="st1")
                nc.tensor.transpose(pt[:], allmax[:, 0:rows], cs["ident"][:])
                tmax = dp.tile([rows, 128], F32, tag="tmax", name="tmax")
                nc.scalar.copy(tmax[:], pt[:])
                rowmax = dp.tile([rows, 1], F32, tag="rowmax", name="rowmax")
                nc.vector.tensor_reduce(rowmax[:], tmax[:], axis=AX.X, op=AT.max)
                prm = pp.tile([1, rows], F32, tag="st2", name="st2")
                nc.tensor.transpose(prm[:], rowmax[:], cs["ident"][0:rows, 0:rows])
                rmT = dp.tile([1, rows], F32, tag="rmT", name="rmT")
                nc.scalar.copy(rmT[:], prm[:])
                pmb = pp.tile([128, rows], F32, tag="inv", name="inv")
                nc.tensor.matmul(pmb[:], lhsT=cs["ones1x128"][:], rhs=rmT[:],
                                 start=True, stop=True)
                Mb = dp.tile([128, rows], F32, tag="Mb", name="Mb")
                nc.scalar.copy(Mb[:], pmb[:])

                eqm = dp.tile([128, min(rows, 8) * 256], BF16, tag="eqm", name="eqm")
                selm = dp.tile([128, min(rows, 8) * 256], F32, tag="selm", name="selm")
                for bb in range(max(1, rows // 8)):
                    csl = bass.ts(bb, min(rows, 8) * 256)
                    nr8 = min(rows, 8)
                    mbb = Mb[:, bb * nr8:(bb + 1) * nr8]\
                        .rearrange("p (a b) -> p a b", b=1)\
                        .to_broadcast([128, nr8, 256])
                    ccv = ccm_all[:, csl].rearrange("p (a b) -> p a b", b=256)
                    nc.vector.tensor_tensor(
                        eqm[:].rearrange("p (a b) -> p a b", b=256),
                        ccv, mbb, op=AT.is_equal)
                    nc.vector.tensor_tensor(
                        selm[:].rearrange("p (a b) -> p a b", b=256),
                        eqm[:].rearrange("p (a b) -> p a b", b=256),
                        _b3(cs["shvB"][:], nr8, 256), op=AT.mult)
                    nc.vector.tensor_reduce(
                        allmin[:, bb * nr8:(bb + 1) * nr8],
                        selm[:].rearrange("p (a b) -> p a b", b=256),
                        axis=AX.X, op=AT.min)
                pt2 = pp.tile([rows, 128], F32, tag="cc", name="cc")
                nc.tensor.transpose(pt2[:], allmin[:, 0:rows], cs["ident"][:])
                tmin = dp.tile([rows, 128], F32, tag="tmin", name="tmin")
                nc.scalar.copy(tmin[:], pt2[:])
                nc.vector.tensor_reduce(shifts[:], tmin[:], axis=AX.X, op=AT.min)
                nc.vector.tensor_scalar_add(shifts[:], shifts[:], BIGL + float(START0))

                # start = (7040 + shift) mod 14337
                m1 = dp.tile([rows, 1], F32, tag="m1", name="m1")
                nc.vector.tensor_scalar(out=m1[:], in0=shifts[:], scalar1=0.0,
                                        scalar2=None, op0=AT.is_lt)
                nc.vector.scalar_tensor_tensor(
                    shifts[:], m1[:], float(CONV_LEN), shifts[:],
                    op0=AT.mult, op1=AT.add)
                nc.vector.tensor_scalar(out=m1[:], in0=shifts[:],
                                        scalar1=float(CONV_LEN), scalar2=None,
                                        op0=AT.is_ge)
                nc.vector.scalar_tensor_tensor(
                    shifts[:], m1[:], float(-CONV_LEN), shifts[:],
                    op0=AT.mult, op1=AT.add)

                idxf = dp.tile([rows, CROP], F32, tag="idxf", name="idxf")
                nc.vector.tensor_tensor(idxf[:], cs["winidx"][0:rows, :],
                                        shifts[:].to_broadcast([rows, CROP]),
                                        op=AT.add)
                idxi = dp.tile([rows, CROP], I32, tag="idxi", name="idxi")
                nc.vector.tensor_copy(idxi[:], idxf[:])
                w = dp.tile([rows, CROP], F32, tag="wg", name="wg")
                nc.gpsimd.indirect_dma_start(
                    out=w[:], out_offset=None,
                    in_=scratch.ap().rearrange("r p -> (r p)").rearrange(
                        "(a b) -> a b", b=1),
                    in_offset=bass.IndirectOffsetOnAxis(ap=idxi[:], axis=0),
                )
                tw_ = dp.tile([rows, CROP], F32, tag="twin", name="twin")
                nc.sync.dma_start(tw_[:], target[:, START0:START0 + CROP])
                nc.vector.tensor_tensor(w[:], w[:], tw_[:], op=AT.subtract)
                convacc = dp.tile([rows, 1], F32, tag="convacc", name="convacc")
                nc.vector.scalar_tensor_tensor(
                    tw_[:], w[:], 1.0, w[:], op0=AT.bypass, op1=AT.mult,
                    accum_out=convacc[:])

                a0 = dp.tile([128, 1], F32, tag="a0", name="a0")
                nc.vector.tensor_reduce(a0[:], astf_acc[:], axis=AX.X, op=AT.add)
                psa = pp.tile([1, 1], F32, tag="st1", name="st1")
                nc.tensor.matmul(psa[:], lhsT=a0[:], rhs=cs["ones128"][:],
                                 start=True, stop=True)
                psc = pp.tile([1, 1], F32, tag="st2", name="st2")
                nc.tensor.matmul(psc[:], lhsT=convacc[:], rhs=cs["ones64"][0:rows, :],
                                 start=True, stop=True)
                nc.scalar.copy(outt[:, 0:1], psa[:])
                nc.scalar.copy(outt[:, 1:2], psc[:])
                nc.sync.dma_start(out[:], outt[:])

    nc.finalize()
    return nc, consts


_CACHE = {}


def get_built(cdt=F32):
    key = str(cdt)
    if key not in _CACHE:
        _CACHE[key] = build_nc(cdt=cdt)
    return _CACHE[key]


LAST_RESULT = {}


def kernel(pred_astf, true_astf, egf, target_waveform):
    import os
    from concourse.bass_utils import run_bass_kernel_spmd
    cdt = BF16 if os.environ.get("CONVALIGN_BF16") == "1" else F32
    nc, consts = get_built(cdt)
    if cdt != F32:
        import ml_dtypes
        from kernel import make_consts as _mk  # noqa
        mmnames = _mm_const_names()
        consts = {k: (v.astype(ml_dtypes.bfloat16) if k in mmnames else v)
                  for k, v in consts.items()}
    pred_astf = np.ascontiguousarray(np.asarray(pred_astf, np.float32))
    true_astf = np.ascontiguousarray(np.asarray(true_astf, np.float32))
    egf = np.ascontiguousarray(np.asarray(egf, np.float32))
    target_waveform = np.ascontiguousarray(
        np.asarray(target_waveform, np.float32))
    B = pred_astf.shape[0]
    per = B // NCORES
    in_maps = []
    for i in range(NCORES):
        sl = slice(i * per, (i + 1) * per)
        m = {"pred": pred_astf[sl], "true": true_astf[sl],
             "egf": egf[sl], "target": target_waveform[sl]}
        m.update(consts)
        in_maps.append(m)
    import os
    trace = os.environ.get("CONVALIGN_TRACE") == "1"
    res = run_bass_kernel_spmd(nc, in_maps, core_ids=list(range(NCORES)),
                               trace=trace)
    LAST_RESULT["res"] = res
    sums = np.stack([res.results[i]["out"][0] for i in range(NCORES)])
    loss_astf = np.float32(sums[:, 0].sum() / (B * L1))
    loss_conv = np.float32(sums[:, 1].sum() / (B * CROP))
    total = np.float32(loss_astf + loss_conv)
    return total, loss_astf, loss_conv



# revision 7
# speedup vs baseline: 1.9478x; 1.4630x over previous
"""Trainium2 Bass kernel for nn_ConvAlignLoss (8-core data parallel).

Self-contained: hardcodes shapes; imports concourse from /opt/trn_rl_repo.

Per core (R=64 rows):
  loss_astf partial: sum((pred-true)^2)
  conv = irfft16384(fft(pred) * conj(fft(egf_pad)))[:14337]  (2-stage matmul FFT)
  cc   = irfft32768(fft(conv_pad) * conj(fft(target_pad)))
  shift = mapped masked argmax of cc (== reference argmax over n=28673)
  loss_conv partial: sum((conv[(7040+i+shift) % 14337] - target[7040+i])^2)
Host combines the 8 cores' (sum_astf, sum_conv) into the scalar losses.

FFT structure (N = 128*N2):
  FWD:  D[t1,t2]=x[N2*t1+t2]; A[t2,f1]=sum_t1 D*W1 (data-stationary matmul);
        B=A*tw; Z[f2,f1]=sum_t2 W2[t2,f2]*B.   Z2d[f2,f1] == X[f1+128*f2]
  INV:  G[f1,t2]=sum_f2 S2d[f2,f1]*V2[f2,t2] (S stationary); H=G*itw;
        x2d[t1,t2]=(1/N) Re(sum_f1 V1[f1,t1]*H[f1,t2])

Perf notes: all matmuls bf16 (1 cyc/row); stage-1/inverse use fused
[Wr|Wi] 256-wide moving operands (half the LDWEIGHTS); twiddle cmuls run
bf16-in/bf16-out in SBUF (DVE 2x_1p) after PSUM->SBUF converting copies
on the scalar/gpsimd engines; DMA batched to one descriptor per tensor
per row-block.
"""
import sys

sys.path.insert(0, "/opt/trn_rl_repo")

import numpy as np
import concourse.bass as bass
import concourse.bacc as bacc
import concourse.mybir as mybir
from concourse import tile

F32 = mybir.dt.float32
BF16 = mybir.dt.bfloat16
I32 = mybir.dt.int32
AT = mybir.AluOpType
AX = mybir.AxisListType

R = 64
NCORES = 8
L1, L2 = 16384, 2048
CONV_LEN = L1 - L2 + 1      # 14337
N_A, N_B = 16384, 32768
GAP_LO, GAP_HI = CONV_LEN, N_B - CONV_LEN + 1   # gap [14337, 18432)
CROP = 256
START0 = (CONV_LEN - CROP) // 2                 # 7040
PITCH = 14592                                   # 57*256
BIGL = float(2 ** 23)


def _dft(n, sign):
    k = np.arange(n)
    return np.exp(sign * 2j * np.pi * np.outer(k, k) / n)


def make_consts():
    c = {}

    def put(name, arr, dt=np.float32):
        c[name] = np.ascontiguousarray(np.asarray(arr, np.float64)).astype(dt)

    W1 = _dft(128, -1)
    put("W1r", W1.real); put("W1i", W1.imag); put("nW1i", -W1.imag)
    put("W1ri", np.concatenate([W1.real, W1.imag], axis=1))      # [128,256]
    twA = np.exp(-2j * np.pi * np.outer(np.arange(128), np.arange(128)) / N_A)
    put("twAr", twA.real); put("twAi", twA.imag)
    V2A = _dft(128, +1)
    put("V2Ari", np.concatenate([V2A.real, V2A.imag], axis=1))   # [128,256]
    put("V2Anri", np.concatenate([-V2A.imag, V2A.real], axis=1))
    itwA = np.exp(2j * np.pi * np.outer(np.arange(128), np.arange(128)) / N_A)
    put("itwAr", itwA.real); put("itwAi", itwA.imag)
    V1A = _dft(128, +1) / N_A
    put("V1Ar", V1A.real); put("nV1Ai", -V1A.imag)

    W2B = _dft(256, -1)          # [t2, f2]
    for a in range(2):
        for b in range(2):
            blk = W2B[a * 128:(a + 1) * 128, b * 128:(b + 1) * 128]
            put(f"W2Br{a}{b}", blk.real)
            put(f"W2Bi{a}{b}", blk.imag)
            put(f"nW2Bi{a}{b}", -blk.imag)
    twB = np.exp(-2j * np.pi * np.outer(np.arange(256), np.arange(128)) / N_B)
    for a in range(2):
        put(f"twBr{a}", twB.real[a * 128:(a + 1) * 128])
        put(f"twBi{a}", twB.imag[a * 128:(a + 1) * 128])
    V2B = _dft(256, +1)          # [f2, t2]
    for a in range(2):
        blk = V2B[a * 128:(a + 1) * 128, :]                      # [128,256]
        put(f"V2Bri{a}", np.concatenate([blk.real, blk.imag], axis=1))
        put(f"V2Bnri{a}", np.concatenate([-blk.imag, blk.real], axis=1))
    itwB = np.exp(2j * np.pi * np.outer(np.arange(128), np.arange(256)) / N_B)
    put("itwBr", itwB.real); put("itwBi", itwB.imag)
    V1B = _dft(128, +1) / N_B
    put("V1Br", V1B.real); put("nV1Bi", -V1B.imag)

    put("ident", np.eye(128))
    put("ones1x128", np.ones((1, 128)))
    put("ones128", np.ones((128, 1)))
    put("ones64", np.ones((64, 1)))

    j = np.arange(128)[:, None] * 256 + np.arange(256)[None, :]   # [t1, t2]
    gap = (j >= GAP_LO) & (j < GAP_HI)
    put("maskB", np.where(gap, -1e30, 0.0))
    shiftval = np.where(j <= CONV_LEN - 1, j - (CONV_LEN - 1), j - GAP_HI + 1)
    put("shvB", np.where(gap, 0.0, shiftval - BIGL))
    put("iota_i", np.tile(np.arange(CROP)[None, :], (R, 1)))      # [64, 256]
    put("rowbase", (np.arange(R) * PITCH)[:, None])               # [64, 1]
    return c


def _b3(ap, n, inner):
    """[128, inner] const AP -> [128, n, inner] broadcast over middle dim."""
    return ap.rearrange("p (a b) -> p a b", a=1).to_broadcast([128, n, inner])


def _cmul_sb(nc, outr, outi, inr, ini, twr, twi, tmp):
    """(outr + i outi) = (inr + i ini) * (twr + i twi); all APs same shape,
    all SBUF bf16 (DVE 2x)."""
    nc.vector.tensor_tensor(outr, inr, twr, op=AT.mult)
    nc.vector.tensor_tensor(tmp, ini, twi, op=AT.mult)
    nc.vector.tensor_tensor(outr, outr, tmp, op=AT.subtract)
    nc.vector.tensor_tensor(outi, inr, twi, op=AT.mult)
    nc.vector.tensor_tensor(tmp, ini, twr, op=AT.mult)
    nc.vector.tensor_tensor(outi, outi, tmp, op=AT.add)


def _mm_const_names():
    s = {"W1r", "W1i", "nW1i", "W1ri", "V2Ari", "V2Anri", "V1Ar", "nV1Ai",
         "itwAr", "itwAi", "twAr", "twAi", "itwBr", "itwBi", "V1Br",
         "nV1Bi"}
    s |= {f"W2Br{a}{b}" for a in range(2) for b in range(2)}
    s |= {f"W2Bi{a}{b}" for a in range(2) for b in range(2)}
    s |= {f"nW2Bi{a}{b}" for a in range(2) for b in range(2)}
    s |= {f"twBr{a}" for a in range(2)} | {f"twBi{a}" for a in range(2)}
    s |= {f"V2Bri{a}" for a in range(2)} | {f"V2Bnri{a}" for a in range(2)}
    return s


def build_nc(rows=R, rbb=8, rb2=4):
    nc = bacc.Bacc("TRN2", target_bir_lowering=False, debug=False,
                   num_devices=NCORES)
    consts = make_consts()

    pred = nc.dram_tensor("pred", [rows, L1], F32, kind="ExternalInput")
    true_ = nc.dram_tensor("true", [rows, L1], F32, kind="ExternalInput")
    egf = nc.dram_tensor("egf", [rows, L2], F32, kind="ExternalInput")
    target = nc.dram_tensor("target", [rows, CONV_LEN], F32,
                            kind="ExternalInput")
    out = nc.dram_tensor("out", [1, 2], F32, kind="ExternalOutput")
    scratch = nc.dram_tensor("scratch", [rows, PITCH], F32)

    MM_CONST = _mm_const_names()

    cdram = {}
    for name, arr in consts.items():
        cdt_n = BF16 if name in MM_CONST else F32
        cdram[name] = nc.dram_tensor(name, list(arr.shape), cdt_n,
                                     kind="ExternalInput")

    nb1, nb2 = rows // rbb, rows // rb2

    with tile.TileContext(nc) as tc:
        with (
            tc.tile_pool(name="consts", bufs=1) as cpool,
            tc.tile_pool(name="keep", bufs=1) as kpool,
            tc.tile_pool(name="ps", bufs=2, space="PSUM") as pp,
        ):
            cs = {}
            for name, arr in consts.items():
                dt = BF16 if name in MM_CONST else F32
                t = cpool.tile(list(arr.shape), dt, tag=f"c_{name}", name=f"c_{name}")
                nc.sync.dma_start(t[:], cdram[name][:])
                cs[name] = t

            allmax = kpool.tile([128, rows], F32, tag="allmax", name="allmax")
            allmin = kpool.tile([128, rows], F32, tag="allmin", name="allmin")
            ccm_all = kpool.tile([128, rows * 256], BF16, tag="ccm", name="ccm")
            astf_acc = kpool.tile([128, nb1], F32, tag="astfacc", name="astfacc")
            shifts = kpool.tile([rows, 1], F32, tag="shifts", name="shifts")
            outt = kpool.tile([1, 2], F32, tag="outt", name="outt")

            # ---------------- B) 16K level (astf fused in) ----------------
            with tc.tile_pool(name="p16", bufs=1) as dp:
                for b in range(nb1):
                    r0 = b * rbb
                    # batched loads (one descriptor each)
                    Dst = dp.tile([128, rbb * 128], F32, tag="Dst", name="Dst")
                    Tst = dp.tile([128, rbb * 128], F32, tag="Tst", name="Tst")
                    Est = dp.tile([16, rbb * 128], F32, tag="Est", name="Est")
                    nc.sync.dma_start(
                        Dst[:].rearrange("p (r c) -> p r c", c=128),
                        pred.ap()[r0:r0 + rbb, :]
                            .rearrange("r (a c) -> a r c", a=128))
                    nc.sync.dma_start(
                        Tst[:].rearrange("p (r c) -> p r c", c=128),
                        true_.ap()[r0:r0 + rbb, :]
                            .rearrange("r (a c) -> a r c", a=128))
                    nc.sync.dma_start(
                        Est[:].rearrange("p (r c) -> p r c", c=128),
                        egf.ap()[r0:r0 + rbb, :]
                            .rearrange("r (a c) -> a r c", a=16))

                    # astf partial: sum((pred-true)^2)
                    nc.vector.tensor_tensor(Tst[:], Dst[:], Tst[:],
                                            op=AT.subtract)
                    nc.vector.scalar_tensor_tensor(
                        Tst[:], Tst[:], 1.0, Tst[:], op0=AT.bypass,
                        op1=AT.mult, accum_out=astf_acc[:, b:b + 1])

                    # bf16 conversions
                    Dp = dp.tile([128, rbb * 128], BF16, tag="Dp", name="Dp")
                    De = dp.tile([16, rbb * 128], BF16, tag="De", name="De")
                    nc.scalar.copy(Dp[:], Dst[:])
                    nc.gpsimd.tensor_copy(De[:], Est[:])

                    # stage 1: A[t2, f1] (fused [Ar|Ai] via W1ri), PSUM 2q/bank
                    Asb = dp.tile([128, rbb * 256], BF16, tag="Asb", name="Asb")
                    Aeb = dp.tile([128, rbb * 256], BF16, tag="Aeb", name="Aeb")
                    for g2 in range(rbb // 2):
                        pa = pp.tile([128, 512], F32, tag="st1", name="st1")
                        pae = pp.tile([128, 512], F32, tag="st1", name="st1")
                        for q in range(2):
                            qq = g2 * 2 + q
                            nc.tensor.matmul(pa[:, bass.ts(q, 256)],
                                             lhsT=Dp[:, bass.ts(qq, 128)],
                                             rhs=cs["W1ri"][:],
                                             start=True, stop=True)
                            nc.tensor.matmul(pae[:, bass.ts(q, 256)],
                                             lhsT=De[:, bass.ts(qq, 128)],
                                             rhs=cs["W1ri"][0:16, :],
                                             start=True, stop=True)
                        nc.scalar.copy(Asb[:, bass.ts(g2, 512)], pa[:])
                        nc.scalar.copy(Aeb[:, bass.ts(g2, 512)], pae[:])

                    # twiddle (bf16, all rows at once): B = A * twA
                    Bs = {k: dp.tile([128, rbb * 128], BF16, tag=f"B{k}",
                                     name=f"B{k}")
                          for k in ("pr", "pi", "er", "ei")}
                    tmpw = dp.tile([128, rbb * 128], BF16, tag="tmpw",
                                   name="tmpw")
                    twr = _b3(cs["twAr"][:], rbb, 128)
                    twi = _b3(cs["twAi"][:], rbb, 128)
                    tmpv = tmpw[:].rearrange("p (q c) -> p q c", c=128)
                    for inp, At in (("p", Asb), ("e", Aeb)):
                        Av = At[:].rearrange("p (q s c) -> p q s c", s=2, c=128)
                        _cmul_sb(nc,
                                 Bs[inp + "r"][:].rearrange(
                                     "p (q c) -> p q c", c=128),
                                 Bs[inp + "i"][:].rearrange(
                                     "p (q c) -> p q c", c=128),
                                 Av[:, :, 0, :], Av[:, :, 1, :],
                                 twr, twi, tmpv)

                    # stage 2: Z[f2, f1]
                    Zs = {k: dp.tile([128, rbb * 128], BF16, tag=f"Z{k}",
                                     name=f"Z{k}")
                          for k in ("pr", "pi", "er", "ei")}
                    for g in range(rbb // 4):
                        gsl = bass.ts(g, 512)
                        for inp in ("p", "e"):
                            br, bi = Bs[inp + "r"], Bs[inp + "i"]
                            pzr = pp.tile([128, 512], F32, tag="st2", name="st2")
                            pzi = pp.tile([128, 512], F32, tag="st2", name="st2")
                            nc.tensor.matmul(pzr[:], lhsT=cs["W1r"][:],
                                             rhs=br[:, gsl], start=True, stop=False)
                            nc.tensor.matmul(pzr[:], lhsT=cs["nW1i"][:],
                                             rhs=bi[:, gsl], start=False, stop=True)
                            nc.tensor.matmul(pzi[:], lhsT=cs["W1i"][:],
                                             rhs=br[:, gsl], start=True, stop=False)
                            nc.tensor.matmul(pzi[:], lhsT=cs["W1r"][:],
                                             rhs=bi[:, gsl], start=False, stop=True)
                            nc.scalar.copy(Zs[inp + "r"][:, gsl], pzr[:])
                            nc.scalar.copy(Zs[inp + "i"][:, gsl], pzi[:])

                    # S = Zp * conj(Ze)   (all bf16 SBUF)
                    Sr = dp.tile([128, rbb * 128], BF16, tag="Sr", name="Sr")
                    Si = dp.tile([128, rbb * 128], BF16, tag="Si", name="Si")
                    tmpb = dp.tile([128, rbb * 128], BF16, tag="tmpbig",
                                   name="tmpbig")
                    nc.vector.tensor_tensor(Sr[:], Zs["pr"][:], Zs["er"][:], op=AT.mult)
                    nc.vector.tensor_tensor(tmpb[:], Zs["pi"][:], Zs["ei"][:], op=AT.mult)
                    nc.vector.tensor_tensor(Sr[:], Sr[:], tmpb[:], op=AT.add)
                    nc.vector.tensor_tensor(Si[:], Zs["pi"][:], Zs["er"][:], op=AT.mult)
                    nc.vector.tensor_tensor(tmpb[:], Zs["pr"][:], Zs["ei"][:], op=AT.mult)
                    nc.vector.tensor_tensor(Si[:], Si[:], tmpb[:], op=AT.subtract)

                    # inverse stage 1: G[f1, t2] (fused [Gr|Gi] via V2Ari)
                    Gsb = dp.tile([128, rbb * 256], BF16, tag="Gsb", name="Gsb")
                    for g2 in range(rbb // 2):
                        pg = pp.tile([128, 512], F32, tag="inv", name="inv")
                        for q in range(2):
                            qq = g2 * 2 + q
                            sl = bass.ts(qq, 128)
                            nc.tensor.matmul(pg[:, bass.ts(q, 256)],
                                             lhsT=Sr[:, sl], rhs=cs["V2Ari"][:],
                                             start=True, stop=False)
                            nc.tensor.matmul(pg[:, bass.ts(q, 256)],
                                             lhsT=Si[:, sl], rhs=cs["V2Anri"][:],
                                             start=False, stop=True)
                        nc.scalar.copy(Gsb[:, bass.ts(g2, 512)], pg[:])

                    # inverse twiddle: H = G * itwA  (bf16)
                    Hr = dp.tile([128, rbb * 128], BF16, tag="Hr", name="Hr")
                    Hi = dp.tile([128, rbb * 128], BF16, tag="Hi", name="Hi")
                    Gv = Gsb[:].rearrange("p (q s c) -> p q s c", s=2, c=128)
                    _cmul_sb(nc,
                             Hr[:].rearrange("p (q c) -> p q c", c=128),
                             Hi[:].rearrange("p (q c) -> p q c", c=128),
                             Gv[:, :, 0, :], Gv[:, :, 1, :],
                             _b3(cs["itwAr"][:], rbb, 128),
                             _b3(cs["itwAi"][:], rbb, 128), tmpv)

                    # final: conv rows (f32 out for scratch)
                    convSB = dp.tile([128, rbb * 128], F32, tag="convSB",
                                     name="convSB")
                    for g in range(rbb // 4):
                        gsl = bass.ts(g, 512)
                        pc = pp.tile([128, 512], F32, tag="cc", name="cc")
                        nc.tensor.matmul(pc[:], lhsT=cs["V1Ar"][:],
                                         rhs=Hr[:, gsl], start=True, stop=False)
                        nc.tensor.matmul(pc[:], lhsT=cs["nV1Ai"][:],
                                         rhs=Hi[:, gsl], start=False, stop=True)
                        nc.scalar.copy(convSB[:, gsl], pc[:])

                    # batched scratch write: conv[0:14336] + elem 14336
                    nc.sync.dma_start(
                        scratch.ap()[r0:r0 + rbb, 0:14336]
                            .rearrange("r (a c) -> a r c", a=112),
                        convSB[0:112, :].rearrange("p (r c) -> p r c", c=128))
                    nc.sync.dma_start(
                        scratch.ap()[r0:r0 + rbb, 14336:14337]
                            .rearrange("r (c d) -> c r d", c=1),
                        convSB[112:113, :]
                            .rearrange("p (r c) -> p r c", c=128)[:, :, 0:1])

            # ---------------- C) 32K level ----------------
            with tc.tile_pool(name="p32", bufs=1) as dp:
                D2c = dp.tile([128, rb2 * 256], BF16, tag="D2c", name="D2c")
                D2t = dp.tile([128, rb2 * 256], BF16, tag="D2t", name="D2t")
                nc.scalar.memzero(D2c[:])
                nc.scalar.memzero(D2t[:])
                tgc = dp.tile([128, rb2 * 256], F32, tag="tgc", name="tgc")
                tgt_ = dp.tile([128, rb2 * 256], F32, tag="tgt", name="tgt")
                nc.scalar.memzero(tgc[:])
                nc.scalar.memzero(tgt_[:])
                for b in range(nb2):
                    r0 = b * rb2
                    nc.sync.dma_start(
                        tgc[0:56, :].rearrange("p (r c) -> p r c", c=256),
                        scratch.ap()[r0:r0 + rb2, 0:14336]
                            .rearrange("r (a c) -> a r c", a=56))
                    nc.sync.dma_start(
                        tgc[56:57, :].rearrange("p (r c) -> p r c", c=256)[:, :, 0:1],
                        scratch.ap()[r0:r0 + rb2, 14336:14337]
                            .rearrange("r (c d) -> c r d", c=1))
                    nc.sync.dma_start(
                        tgt_[0:56, :].rearrange("p (r c) -> p r c", c=256),
                        target.ap()[r0:r0 + rb2, 0:14336]
                            .rearrange("r (a c) -> a r c", a=56))
                    nc.sync.dma_start(
                        tgt_[56:57, :].rearrange("p (r c) -> p r c", c=256)[:, :, 0:1],
                        target.ap()[r0:r0 + rb2, 14336:14337]
                            .rearrange("r (c d) -> c r d", c=1))
                    # staging rows 0:56 fully rewritten; row 56 keeps zeros
                    # outside the q*256 slots (zeroed once before the loop)
                    nc.scalar.copy(D2c[0:57, :], tgc[0:57, :])
                    nc.scalar.copy(D2t[0:57, :], tgt_[0:57, :])

                    # stage 1: A[t2, f1] per q, both inputs; fused [Ar|Ai]
                    A2c = dp.tile([128, rb2 * 512], BF16, tag="A2c", name="A2c")
                    A2t = dp.tile([128, rb2 * 512], BF16, tag="A2t", name="A2t")
                    for q in range(rb2):
                        pac = pp.tile([128, 512], F32, tag="st1", name="st1")
                        pat = pp.tile([128, 512], F32, tag="st1", name="st1")
                        for cch in range(2):
                            dsl = slice(q * 256 + cch * 128,
                                        q * 256 + cch * 128 + 128)
                            nc.tensor.matmul(pac[:, bass.ts(cch, 256)],
                                             lhsT=D2c[0:57, dsl],
                                             rhs=cs["W1ri"][0:57, :],
                                             start=True, stop=True)
                            nc.tensor.matmul(pat[:, bass.ts(cch, 256)],
                                             lhsT=D2t[0:57, dsl],
                                             rhs=cs["W1ri"][0:57, :],
                                             start=True, stop=True)
                        nc.scalar.copy(A2c[:, bass.ts(q, 512)], pac[:])
                        nc.scalar.copy(A2t[:, bass.ts(q, 512)], pat[:])

                    # twiddle per t2-chunk: B = A * twB{c}
                    B2 = {}
                    tmp2 = dp.tile([128, rb2 * 128], BF16, tag="tmp2",
                                   name="tmp2")
                    tmp2v = tmp2[:].rearrange("p (q c) -> p q c", c=128)
                    for inp, At in (("c", A2c), ("t", A2t)):
                        Av = At[:].rearrange("p (q h s c) -> p q h s c",
                                             h=2, s=2, c=128)
                        for cch in range(2):
                            br = dp.tile([128, rb2 * 128], BF16,
                                         tag=f"B2r{cch}{inp}", name=f"B2r{cch}{inp}")
                            bi = dp.tile([128, rb2 * 128], BF16,
                                         tag=f"B2i{cch}{inp}", name=f"B2i{cch}{inp}")
                            _cmul_sb(nc,
                                     br[:].rearrange("p (q c) -> p q c", c=128),
                                     bi[:].rearrange("p (q c) -> p q c", c=128),
                                     Av[:, :, cch, 0, :], Av[:, :, cch, 1, :],
                                     _b3(cs[f"twBr{cch}"][:], rb2, 128),
                                     _b3(cs[f"twBi{cch}"][:], rb2, 128),
                                     tmp2v)
                            B2[(cch, inp)] = (br, bi)

                    # stage 2: Z[f2, f1]
                    Z2 = {}
                    for inp in ("c", "t"):
                        for f2c in range(2):
                            zr = dp.tile([128, rb2 * 128], BF16,
                                         tag=f"Z2r{inp}{f2c}", name=f"Z2r{inp}{f2c}")
                            zi = dp.tile([128, rb2 * 128], BF16,
                                         tag=f"Z2i{inp}{f2c}", name=f"Z2i{inp}{f2c}")
                            pzr = pp.tile([128, rb2 * 128], F32, tag="st2", name="st2")
                            pzi = pp.tile([128, rb2 * 128], F32, tag="st2", name="st2")
                            for t2c in range(2):
                                br, bi = B2[(t2c, inp)]
                                nc.tensor.matmul(pzr[:], lhsT=cs[f"W2Br{t2c}{f2c}"][:],
                                                 rhs=br[:], start=(t2c == 0), stop=False)
                                nc.tensor.matmul(pzr[:], lhsT=cs[f"nW2Bi{t2c}{f2c}"][:],
                                                 rhs=bi[:], start=False, stop=(t2c == 1))
                                nc.tensor.matmul(pzi[:], lhsT=cs[f"W2Bi{t2c}{f2c}"][:],
                                                 rhs=br[:], start=(t2c == 0), stop=False)
                                nc.tensor.matmul(pzi[:], lhsT=cs[f"W2Br{t2c}{f2c}"][:],
                                                 rhs=bi[:], start=False, stop=(t2c == 1))
                            nc.scalar.copy(zr[:], pzr[:])
                            nc.scalar.copy(zi[:], pzi[:])
                            Z2[(inp, f2c)] = (zr, zi)

                    # S = Zc * conj(Zt)  (bf16)
                    S2 = {}
                    tmpc = dp.tile([128, rb2 * 128], BF16, tag="tmpc", name="tmpc")
                    for f2c in range(2):
                        zcr, zci = Z2[("c", f2c)]
                        ztr, zti = Z2[("t", f2c)]
                        sr = dp.tile([128, rb2 * 128], BF16, tag=f"S2r{f2c}",
                                     name=f"S2r{f2c}")
                        si = dp.tile([128, rb2 * 128], BF16, tag=f"S2i{f2c}",
                                     name=f"S2i{f2c}")
                        nc.vector.tensor_tensor(sr[:], zcr[:], ztr[:], op=AT.mult)
                        nc.vector.tensor_tensor(tmpc[:], zci[:], zti[:], op=AT.mult)
                        nc.vector.tensor_tensor(sr[:], sr[:], tmpc[:], op=AT.add)
                        nc.vector.tensor_tensor(si[:], zci[:], ztr[:], op=AT.mult)
                        nc.vector.tensor_tensor(tmpc[:], zcr[:], zti[:], op=AT.mult)
                        nc.vector.tensor_tensor(si[:], si[:], tmpc[:], op=AT.subtract)
                        S2[f2c] = (sr, si)

                    # inverse stage 1: G[f1, t2] fused [Gr(256)|Gi(256)] per q
                    G2sb = dp.tile([128, rb2 * 512], BF16, tag="G2sb", name="G2sb")
                    for q in range(rb2):
                        pg = pp.tile([128, 512], F32, tag="inv", name="inv")
                        sl = bass.ts(q, 128)
                        for f2c in range(2):
                            sr, si = S2[f2c]
                            nc.tensor.matmul(pg[:], lhsT=sr[:, sl],
                                             rhs=cs[f"V2Bri{f2c}"][:],
                                             start=(f2c == 0), stop=False)
                            nc.tensor.matmul(pg[:], lhsT=si[:, sl],
                                             rhs=cs[f"V2Bnri{f2c}"][:],
                                             start=False, stop=(f2c == 1))
                        nc.scalar.copy(G2sb[:, bass.ts(q, 512)], pg[:])

                    # inverse twiddle: H = G * itwB (bf16)
                    H2r = dp.tile([128, rb2 * 256], BF16, tag="H2r", name="H2r")
                    H2i = dp.tile([128, rb2 * 256], BF16, tag="H2i", name="H2i")
                    G2v = G2sb[:].rearrange("p (q s c) -> p q s c", s=2, c=256)
                    tmph = dp.tile([128, rb2 * 256], BF16, tag="tmph", name="tmph")
                    _cmul_sb(nc,
                             H2r[:].rearrange("p (q c) -> p q c", c=256),
                             H2i[:].rearrange("p (q c) -> p q c", c=256),
                             G2v[:, :, 0, :], G2v[:, :, 1, :],
                             _b3(cs["itwBr"][:], rb2, 256),
                             _b3(cs["itwBi"][:], rb2, 256),
                             tmph[:].rearrange("p (q c) -> p q c", c=256))

                    # final: cc rows + mask + per-(t1,row) max
                    for g in range(rb2 // 2):
                        gsl = bass.ts(g, 512)
                        pcc = pp.tile([128, 512], F32, tag="cc", name="cc")
                        nc.tensor.matmul(pcc[:], lhsT=cs["V1Br"][:],
                                         rhs=H2r[:, gsl], start=True, stop=False)
                        nc.tensor.matmul(pcc[:], lhsT=cs["nV1Bi"][:],
                                         rhs=H2i[:, gsl], start=False, stop=True)
                        csl = slice((r0 + g * 2) * 256, (r0 + g * 2 + 2) * 256)
                        ccv = ccm_all[:, csl].rearrange("p (a b) -> p a b", b=256)
                        nc.vector.scalar_tensor_tensor(
                            ccv, pcc[:].rearrange("p (a b) -> p a b", b=256),
                            1.0, _b3(cs["maskB"][:], 2, 256),
                            op0=AT.bypass, op1=AT.add)
                        nc.vector.tensor_reduce(
                            allmax[:, r0 + g * 2:r0 + g * 2 + 2], ccv,
                            axis=AX.X, op=AT.max)

            # ---------------- D) argmax -> shifts -> loss ----------------
            with tc.tile_pool(name="amax", bufs=1) as dp:
                pt = pp.tile([rows, 128], F32, tag="st1", name="st1")
                nc.tensor.transpose(pt[:], allmax[:, 0:rows], cs["ident"][:])
                tmax = dp.tile([rows, 128], F32, tag="tmax", name="tmax")
                nc.scalar.copy(tmax[:], pt[:])
                rowmax = dp.tile([rows, 1], F32, tag="rowmax", name="rowmax")
                nc.vector.tensor_reduce(rowmax[:], tmax[:], axis=AX.X, op=AT.max)
                prm = pp.tile([1, rows], F32, tag="st2", name="st2")
                nc.tensor.transpose(prm[:], rowmax[:], cs["ident"][0:rows, 0:rows])
                rmT = dp.tile([1, rows], F32, tag="rmT", name="rmT")
                nc.scalar.copy(rmT[:], prm[:])
                pmb = pp.tile([128, rows], F32, tag="inv", name="inv")
                nc.tensor.matmul(pmb[:], lhsT=cs["ones1x128"][:], rhs=rmT[:],
                                 start=True, stop=True)
                Mb = dp.tile([128, rows], F32, tag="Mb", name="Mb")
                nc.scalar.copy(Mb[:], pmb[:])

                eqm = dp.tile([128, min(rows, 8) * 256], BF16, tag="eqm", name="eqm")
                selm = dp.tile([128, min(rows, 8) * 256], F32, tag="selm", name="selm")
                for bb in range(max(1, rows // 8)):
                    csl = bass.ts(bb, min(rows, 8) * 256)
                    nr8 = min(rows, 8)
                    mbb = Mb[:, bb * nr8:(bb + 1) * nr8]\
                        .rearrange("p (a b) -> p a b", b=1)\
                        .to_broadcast([128, nr8, 256])
                    ccv = ccm_all[:, csl].rearrange("p (a b) -> p a b", b=256)
                    nc.vector.tensor_tensor(
                        eqm[:].rearrange("p (a b) -> p a b", b=256),
                        ccv, mbb, op=AT.is_equal)
                    nc.vector.tensor_tensor(
                        selm[:].rearrange("p (a b) -> p a b", b=256),
                        eqm[:].rearrange("p (a b) -> p a b", b=256),
                        _b3(cs["shvB"][:], nr8, 256), op=AT.mult)
                    nc.vector.tensor_reduce(
                        allmin[:, bb * nr8:(bb + 1) * nr8],
                        selm[:].rearrange("p (a b) -> p a b", b=256),
                        axis=AX.X, op=AT.min)
                pt2 = pp.tile([rows, 128], F32, tag="cc", name="cc")
                nc.tensor.transpose(pt2[:], allmin[:, 0:rows], cs["ident"][:])
                tmin = dp.tile([rows, 128], F32, tag="tmin", name="tmin")
                nc.scalar.copy(tmin[:], pt2[:])
                nc.vector.tensor_reduce(shifts[:], tmin[:], axis=AX.X, op=AT.min)
                nc.vector.tensor_scalar_add(shifts[:], shifts[:], BIGL + float(START0))

                # start = (7040 + shift) mod 14337
                m1 = dp.tile([rows, 1], F32, tag="m1", name="m1")
                nc.vector.tensor_scalar(out=m1[:], in0=shifts[:], scalar1=0.0,
                                        scalar2=None, op0=AT.is_lt)
                nc.vector.scalar_tensor_tensor(
                    shifts[:], m1[:], float(CONV_LEN), shifts[:],
                    op0=AT.mult, op1=AT.add)
                nc.vector.tensor_scalar(out=m1[:], in0=shifts[:],
                                        scalar1=float(CONV_LEN), scalar2=None,
                                        op0=AT.is_ge)
                nc.vector.scalar_tensor_tensor(
                    shifts[:], m1[:], float(-CONV_LEN), shifts[:],
                    op0=AT.mult, op1=AT.add)

                # idx[r, i] = r*PITCH + (start + i) mod 14337
                idxf = dp.tile([rows, CROP], F32, tag="idxf", name="idxf")
                m2 = dp.tile([rows, CROP], F32, tag="m2", name="m2")
                nc.vector.tensor_tensor(idxf[:], cs["iota_i"][0:rows, :],
                                        shifts[:].to_broadcast([rows, CROP]),
                                        op=AT.add)
                nc.vector.tensor_scalar(out=m2[:], in0=idxf[:],
                                        scalar1=float(CONV_LEN), scalar2=None,
                                        op0=AT.is_ge)
                nc.vector.scalar_tensor_tensor(
                    idxf[:], m2[:], float(-CONV_LEN), idxf[:],
                    op0=AT.mult, op1=AT.add)
                nc.vector.tensor_tensor(
                    idxf[:], idxf[:],
                    cs["rowbase"][0:rows, :].to_broadcast([rows, CROP]),
                    op=AT.add)
                idxi = dp.tile([rows, CROP], I32, tag="idxi", name="idxi")
                nc.vector.tensor_copy(idxi[:], idxf[:])
                w = dp.tile([rows, CROP], F32, tag="wg", name="wg")
                nc.gpsimd.indirect_dma_start(
                    out=w[:], out_offset=None,
                    in_=scratch.ap().rearrange("r p -> (r p)").rearrange(
                        "(a b) -> a b", b=1),
                    in_offset=bass.IndirectOffsetOnAxis(ap=idxi[:], axis=0),
                )
                tw_ = dp.tile([rows, CROP], F32, tag="twin", name="twin")
                nc.sync.dma_start(tw_[:], target[:, START0:START0 + CROP])
                nc.vector.tensor_tensor(w[:], w[:], tw_[:], op=AT.subtract)
                convacc = dp.tile([rows, 1], F32, tag="convacc", name="convacc")
                nc.vector.scalar_tensor_tensor(
                    tw_[:], w[:], 1.0, w[:], op0=AT.bypass, op1=AT.mult,
                    accum_out=convacc[:])

                a0 = dp.tile([128, 1], F32, tag="a0", name="a0")
                nc.vector.tensor_reduce(a0[:], astf_acc[:], axis=AX.X, op=AT.add)
                psa = pp.tile([1, 1], F32, tag="st1", name="st1")
                nc.tensor.matmul(psa[:], lhsT=a0[:], rhs=cs["ones128"][:],
                                 start=True, stop=True)
                psc = pp.tile([1, 1], F32, tag="st2", name="st2")
                nc.tensor.matmul(psc[:], lhsT=convacc[:], rhs=cs["ones64"][0:rows, :],
                                 start=True, stop=True)
                nc.scalar.copy(outt[:, 0:1], psa[:])
                nc.scalar.copy(outt[:, 1:2], psc[:])
                nc.sync.dma_start(out[:], outt[:])

    nc.finalize()
    return nc, consts


_CACHE = {}


def get_built():
    if "nc" not in _CACHE:
        _CACHE["nc"] = build_nc()
    return _CACHE["nc"]


LAST_RESULT = {}


def kernel(pred_astf, true_astf, egf, target_waveform):
    import os
    import ml_dtypes
    from concourse.bass_utils import run_bass_kernel_spmd
    nc, consts = get_built()
    mmnames = _mm_const_names()
    consts = {k: (v.astype(ml_dtypes.bfloat16) if k in mmnames else v)
              for k, v in consts.items()}
    pred_astf = np.ascontiguousarray(np.asarray(pred_astf, np.float32))
    true_astf = np.ascontiguousarray(np.asarray(true_astf, np.float32))
    egf = np.ascontiguousarray(np.asarray(egf, np.float32))
    target_waveform = np.ascontiguousarray(
        np.asarray(target_waveform, np.float32))
    B = pred_astf.shape[0]
    per = B // NCORES
    in_maps = []
    for i in range(NCORES):
        sl = slice(i * per, (i + 1) * per)
        m = {"pred": pred_astf[sl], "true": true_astf[sl],
             "egf": egf[sl], "target": target_waveform[sl]}
        m.update(consts)
        in_maps.append(m)
    trace = os.environ.get("CONVALIGN_TRACE") == "1"
    res = run_bass_kernel_spmd(nc, in_maps, core_ids=list(range(NCORES)),
                               trace=trace)
    LAST_RESULT["res"] = res
    sums = np.stack([res.results[i]["out"][0] for i in range(NCORES)])
    loss_astf = np.float32(sums[:, 0].sum() / (B * L1))
    loss_conv = np.float32(sums[:, 1].sum() / (B * CROP))
    total = np.float32(loss_astf + loss_conv)
    return total, loss_astf, loss_conv


# revision 9
# speedup vs baseline: 2.2906x; 1.1760x over previous
"""Trainium2 Bass kernel for nn_ConvAlignLoss (8-core data parallel).

Self-contained: hardcodes shapes; imports concourse from /opt/trn_rl_repo.

Per core (R=64 rows):
  loss_astf partial: sum((pred-true)^2)
  conv = irfft16384(fft(pred) * conj(fft(egf_pad)))[:14337]  (2-stage matmul FFT)
  cc   = irfft32768(fft(conv_pad) * conj(fft(target_pad)))
  shift = mapped masked argmax of cc (== reference argmax over n=28673)
  loss_conv partial: sum((conv[(7040+i+shift) % 14337] - target[7040+i])^2)
Host combines the 8 cores' (sum_astf, sum_conv) into the scalar losses.

FFT structure (N = 128*N2):
  FWD:  D[t1,t2]=x[N2*t1+t2]; A[t2,f1]=sum_t1 D*W1 (data-stationary matmul);
        B=A*tw; Z[f2,f1]=sum_t2 W2[t2,f2]*B.   Z2d[f2,f1] == X[f1+128*f2]
  INV:  G[f1,t2]=sum_f2 S2d[f2,f1]*V2[f2,t2] (S stationary); H=G*itw;
        x2d[t1,t2]=(1/N) Re(sum_f1 V1[f1,t1]*H[f1,t2])

Perf notes: all matmuls bf16 (1 cyc/row); stage-1/inverse use fused
[Wr|Wi] 256-wide moving operands (half the LDWEIGHTS); twiddle cmuls run
bf16-in/bf16-out in SBUF (DVE 2x_1p) after PSUM->SBUF converting copies
on the scalar/gpsimd engines; DMA batched to one descriptor per tensor
per row-block.
"""
import sys

sys.path.insert(0, "/opt/trn_rl_repo")

import numpy as np
import concourse.bass as bass
import concourse.bacc as bacc
import concourse.mybir as mybir
from concourse import tile

F32 = mybir.dt.float32
BF16 = mybir.dt.bfloat16
I32 = mybir.dt.int32
AT = mybir.AluOpType
AX = mybir.AxisListType

R = 64
NCORES = 8
L1, L2 = 16384, 2048
CONV_LEN = L1 - L2 + 1      # 14337
N_A, N_B = 16384, 32768
GAP_LO, GAP_HI = CONV_LEN, N_B - CONV_LEN + 1   # gap [14337, 18432)
CROP = 256
START0 = (CONV_LEN - CROP) // 2                 # 7040
PITCH = 14592                                   # 57*256
BIGL = float(2 ** 23)


def _dft(n, sign):
    k = np.arange(n)
    return np.exp(sign * 2j * np.pi * np.outer(k, k) / n)


def make_consts():
    c = {}

    def put(name, arr, dt=np.float32):
        c[name] = np.ascontiguousarray(np.asarray(arr, np.float64)).astype(dt)

    W1 = _dft(128, -1)
    put("W1r", W1.real); put("W1i", W1.imag); put("nW1i", -W1.imag)
    put("W1ri", np.concatenate([W1.real, W1.imag], axis=1))      # [128,256]
    twA = np.exp(-2j * np.pi * np.outer(np.arange(128), np.arange(128)) / N_A)
    put("twAr", twA.real); put("twAi", twA.imag)
    V2A = _dft(128, +1)
    put("V2Ari", np.concatenate([V2A.real, V2A.imag], axis=1))   # [128,256]
    put("V2Anri", np.concatenate([-V2A.imag, V2A.real], axis=1))
    itwA = np.exp(2j * np.pi * np.outer(np.arange(128), np.arange(128)) / N_A)
    put("itwAr", itwA.real); put("itwAi", itwA.imag)
    V1A = _dft(128, +1) / N_A
    put("V1Ar", V1A.real); put("nV1Ai", -V1A.imag)

    W2B = _dft(256, -1)          # [t2, f2]
    for a in range(2):
        for b in range(2):
            blk = W2B[a * 128:(a + 1) * 128, b * 128:(b + 1) * 128]
            put(f"W2Br{a}{b}", blk.real)
            put(f"W2Bi{a}{b}", blk.imag)
            put(f"nW2Bi{a}{b}", -blk.imag)
    twB = np.exp(-2j * np.pi * np.outer(np.arange(256), np.arange(128)) / N_B)
    for a in range(2):
        put(f"twBr{a}", twB.real[a * 128:(a + 1) * 128])
        put(f"twBi{a}", twB.imag[a * 128:(a + 1) * 128])
    V2B = _dft(256, +1)          # [f2, t2]
    for a in range(2):
        blk = V2B[a * 128:(a + 1) * 128, :]                      # [128,256]
        put(f"V2Bri{a}", np.concatenate([blk.real, blk.imag], axis=1))
        put(f"V2Bnri{a}", np.concatenate([-blk.imag, blk.real], axis=1))
    itwB = np.exp(2j * np.pi * np.outer(np.arange(128), np.arange(256)) / N_B)
    put("itwBr", itwB.real); put("itwBi", itwB.imag)
    V1B = _dft(128, +1) / N_B
    put("V1Br", V1B.real); put("nV1Bi", -V1B.imag)

    put("ident", np.eye(128))
    put("ones1x128", np.ones((1, 128)))
    put("ones128", np.ones((128, 1)))
    put("ones64", np.ones((64, 1)))

    j = np.arange(128)[:, None] * 256 + np.arange(256)[None, :]   # [t1, t2]
    gap = (j >= GAP_LO) & (j < GAP_HI)
    put("maskB", np.where(gap, -1e30, 0.0))
    shiftval = np.where(j <= CONV_LEN - 1, j - (CONV_LEN - 1), j - GAP_HI + 1)
    put("shvB", np.where(gap, 0.0, shiftval - BIGL))
    put("iota_i", np.tile(np.arange(CROP)[None, :], (R, 1)))      # [64, 256]
    put("rowbase", (np.arange(R) * PITCH)[:, None])               # [64, 1]
    return c


def _b3(ap, n, inner):
    """[128, inner] const AP -> [128, n, inner] broadcast over middle dim."""
    return ap.rearrange("p (a b) -> p a b", a=1).to_broadcast([128, n, inner])


def _cmul_sb(nc, outr, outi, inr, ini, twr, twi, tmp):
    """(outr + i outi) = (inr + i ini) * (twr + i twi); all APs same shape,
    all SBUF bf16 (DVE 2x)."""
    nc.vector.tensor_tensor(outr, inr, twr, op=AT.mult)
    nc.vector.tensor_tensor(tmp, ini, twi, op=AT.mult)
    nc.vector.tensor_tensor(outr, outr, tmp, op=AT.subtract)
    nc.vector.tensor_tensor(outi, inr, twi, op=AT.mult)
    nc.vector.tensor_tensor(tmp, ini, twr, op=AT.mult)
    nc.vector.tensor_tensor(outi, outi, tmp, op=AT.add)


def _mm_const_names():
    s = {"W1r", "W1i", "nW1i", "W1ri", "V2Ari", "V2Anri", "V1Ar", "nV1Ai",
         "itwAr", "itwAi", "twAr", "twAi", "itwBr", "itwBi", "V1Br",
         "nV1Bi"}
    s |= {f"W2Br{a}{b}" for a in range(2) for b in range(2)}
    s |= {f"W2Bi{a}{b}" for a in range(2) for b in range(2)}
    s |= {f"nW2Bi{a}{b}" for a in range(2) for b in range(2)}
    s |= {f"twBr{a}" for a in range(2)} | {f"twBi{a}" for a in range(2)}
    s |= {f"V2Bri{a}" for a in range(2)} | {f"V2Bnri{a}" for a in range(2)}
    return s


def build_nc(rows=R, rbb=8, rb2=4):
    nc = bacc.Bacc("TRN2", target_bir_lowering=False, debug=False,
                   num_devices=NCORES)
    consts = make_consts()

    pred = nc.dram_tensor("pred", [rows, L1], F32, kind="ExternalInput")
    true_ = nc.dram_tensor("true", [rows, L1], F32, kind="ExternalInput")
    egf = nc.dram_tensor("egf", [rows, L2], F32, kind="ExternalInput")
    target = nc.dram_tensor("target", [rows, CONV_LEN], F32,
                            kind="ExternalInput")
    out = nc.dram_tensor("out", [1, 2], F32, kind="ExternalOutput")
    scratch = nc.dram_tensor("scratch", [rows, PITCH], F32)

    MM_CONST = _mm_const_names()

    cdram = {}
    for name, arr in consts.items():
        cdt_n = BF16 if name in MM_CONST else F32
        cdram[name] = nc.dram_tensor(name, list(arr.shape), cdt_n,
                                     kind="ExternalInput")

    nb1, nb2 = rows // rbb, rows // rb2

    with tile.TileContext(nc) as tc:
        with (
            tc.tile_pool(name="consts", bufs=1) as cpool,
            tc.tile_pool(name="keep", bufs=1) as kpool,
            tc.tile_pool(name="ps", bufs=2, space="PSUM") as pp,
        ):
            cs = {}
            for name, arr in consts.items():
                dt = BF16 if name in MM_CONST else F32
                t = cpool.tile(list(arr.shape), dt, tag=f"c_{name}", name=f"c_{name}")
                nc.sync.dma_start(t[:], cdram[name][:])
                cs[name] = t

            allmax = kpool.tile([128, rows], F32, tag="allmax", name="allmax")
            allmin = kpool.tile([128, rows], F32, tag="allmin", name="allmin")
            ccm_all = kpool.tile([128, rows * 256], BF16, tag="ccm", name="ccm")
            astf_acc = kpool.tile([128, nb1], F32, tag="astfacc", name="astfacc")
            shifts = kpool.tile([rows, 1], F32, tag="shifts", name="shifts")
            outt = kpool.tile([1, 2], F32, tag="outt", name="outt")

            # ---------------- B) 16K level (astf fused in) ----------------
            with tc.tile_pool(name="p16", bufs=2) as dp:
                for b in range(nb1):
                    r0 = b * rbb
                    # batched loads (one descriptor each)
                    Dst = dp.tile([128, rbb * 128], F32, tag="Dst", name="Dst")
                    Tst = dp.tile([128, rbb * 128], F32, tag="Tst", name="Tst")
                    Est = dp.tile([16, rbb * 128], F32, tag="Est", name="Est")
                    nc.sync.dma_start(
                        Dst[:].rearrange("p (r c) -> p r c", c=128),
                        pred.ap()[r0:r0 + rbb, :]
                            .rearrange("r (a c) -> a r c", a=128))
                    nc.sync.dma_start(
                        Tst[:].rearrange("p (r c) -> p r c", c=128),
                        true_.ap()[r0:r0 + rbb, :]
                            .rearrange("r (a c) -> a r c", a=128))
                    nc.sync.dma_start(
                        Est[:].rearrange("p (r c) -> p r c", c=128),
                        egf.ap()[r0:r0 + rbb, :]
                            .rearrange("r (a c) -> a r c", a=16))

                    # astf partial: sum((pred-true)^2)
                    nc.vector.tensor_tensor(Tst[:], Dst[:], Tst[:],
                                            op=AT.subtract)
                    nc.vector.scalar_tensor_tensor(
                        Tst[:], Tst[:], 1.0, Tst[:], op0=AT.bypass,
                        op1=AT.mult, accum_out=astf_acc[:, b:b + 1])

                    # bf16 conversions
                    Dp = dp.tile([128, rbb * 128], BF16, tag="Dp", name="Dp")
                    De = dp.tile([16, rbb * 128], BF16, tag="De", name="De")
                    nc.scalar.copy(Dp[:], Dst[:])
                    nc.gpsimd.tensor_copy(De[:], Est[:])

                    # stage 1: A[t2, f1] (fused [Ar|Ai] via W1ri), PSUM 2q/bank
                    Asb = dp.tile([128, rbb * 256], BF16, tag="Asb", name="Asb")
                    Aeb = dp.tile([128, rbb * 256], BF16, tag="Aeb", name="Aeb")
                    for g2 in range(rbb // 2):
                        pa = pp.tile([128, 512], F32, tag="st1", name="st1")
                        pae = pp.tile([128, 512], F32, tag="st1", name="st1")
                        for q in range(2):
                            qq = g2 * 2 + q
                            nc.tensor.matmul(pa[:, bass.ts(q, 256)],
                                             lhsT=Dp[:, bass.ts(qq, 128)],
                                             rhs=cs["W1ri"][:],
                                             start=True, stop=True)
                            nc.tensor.matmul(pae[:, bass.ts(q, 256)],
                                             lhsT=De[:, bass.ts(qq, 128)],
                                             rhs=cs["W1ri"][0:16, :],
                                             start=True, stop=True)
                        nc.scalar.copy(Asb[:, bass.ts(g2, 512)], pa[:])
                        nc.scalar.copy(Aeb[:, bass.ts(g2, 512)], pae[:])

                    # twiddle (bf16, all rows at once): B = A * twA
                    Bs = {k: dp.tile([128, rbb * 128], BF16, tag=f"B{k}",
                                     name=f"B{k}")
                          for k in ("pr", "pi", "er", "ei")}
                    tmpw = dp.tile([128, rbb * 128], BF16, tag="tmpw",
                                   name="tmpw")
                    twr = _b3(cs["twAr"][:], rbb, 128)
                    twi = _b3(cs["twAi"][:], rbb, 128)
                    tmpv = tmpw[:].rearrange("p (q c) -> p q c", c=128)
                    for inp, At in (("p", Asb), ("e", Aeb)):
                        Av = At[:].rearrange("p (q s c) -> p q s c", s=2, c=128)
                        _cmul_sb(nc,
                                 Bs[inp + "r"][:].rearrange(
                                     "p (q c) -> p q c", c=128),
                                 Bs[inp + "i"][:].rearrange(
                                     "p (q c) -> p q c", c=128),
                                 Av[:, :, 0, :], Av[:, :, 1, :],
                                 twr, twi, tmpv)

                    # stage 2: Z[f2, f1]
                    Zs = {k: dp.tile([128, rbb * 128], BF16, tag=f"Z{k}",
                                     name=f"Z{k}")
                          for k in ("pr", "pi", "er", "ei")}
                    for g in range(rbb // 4):
                        gsl = bass.ts(g, 512)
                        for inp in ("p", "e"):
                            br, bi = Bs[inp + "r"], Bs[inp + "i"]
                            pzr = pp.tile([128, 512], F32, tag="st2", name="st2")
                            pzi = pp.tile([128, 512], F32, tag="st2", name="st2")
                            nc.tensor.matmul(pzr[:], lhsT=cs["W1r"][:],
                                             rhs=br[:, gsl], start=True, stop=False)
                            nc.tensor.matmul(pzr[:], lhsT=cs["nW1i"][:],
                                             rhs=bi[:, gsl], start=False, stop=True)
                            nc.tensor.matmul(pzi[:], lhsT=cs["W1i"][:],
                                             rhs=br[:, gsl], start=True, stop=False)
                            nc.tensor.matmul(pzi[:], lhsT=cs["W1r"][:],
                                             rhs=bi[:, gsl], start=False, stop=True)
                            nc.scalar.copy(Zs[inp + "r"][:, gsl], pzr[:])
                            nc.scalar.copy(Zs[inp + "i"][:, gsl], pzi[:])

                    # S = Zp * conj(Ze)   (all bf16 SBUF)
                    Sr = dp.tile([128, rbb * 128], BF16, tag="Sr", name="Sr")
                    Si = dp.tile([128, rbb * 128], BF16, tag="Si", name="Si")
                    tmpb = dp.tile([128, rbb * 128], BF16, tag="tmpbig",
                                   name="tmpbig")
                    nc.vector.tensor_tensor(Sr[:], Zs["pr"][:], Zs["er"][:], op=AT.mult)
                    nc.vector.tensor_tensor(tmpb[:], Zs["pi"][:], Zs["ei"][:], op=AT.mult)
                    nc.vector.tensor_tensor(Sr[:], Sr[:], tmpb[:], op=AT.add)
                    nc.vector.tensor_tensor(Si[:], Zs["pi"][:], Zs["er"][:], op=AT.mult)
                    nc.vector.tensor_tensor(tmpb[:], Zs["pr"][:], Zs["ei"][:], op=AT.mult)
                    nc.vector.tensor_tensor(Si[:], Si[:], tmpb[:], op=AT.subtract)

                    # inverse stage 1: G[f1, t2] (fused [Gr|Gi] via V2Ari)
                    Gsb = dp.tile([128, rbb * 256], BF16, tag="Gsb", name="Gsb")
                    for g2 in range(rbb // 2):
                        pg = pp.tile([128, 512], F32, tag="inv", name="inv")
                        for q in range(2):
                            qq = g2 * 2 + q
                            sl = bass.ts(qq, 128)
                            nc.tensor.matmul(pg[:, bass.ts(q, 256)],
                                             lhsT=Sr[:, sl], rhs=cs["V2Ari"][:],
                                             start=True, stop=False)
                            nc.tensor.matmul(pg[:, bass.ts(q, 256)],
                                             lhsT=Si[:, sl], rhs=cs["V2Anri"][:],
                                             start=False, stop=True)
                        nc.scalar.copy(Gsb[:, bass.ts(g2, 512)], pg[:])

                    # inverse twiddle: H = G * itwA  (bf16)
                    Hr = dp.tile([128, rbb * 128], BF16, tag="Hr", name="Hr")
                    Hi = dp.tile([128, rbb * 128], BF16, tag="Hi", name="Hi")
                    Gv = Gsb[:].rearrange("p (q s c) -> p q s c", s=2, c=128)
                    _cmul_sb(nc,
                             Hr[:].rearrange("p (q c) -> p q c", c=128),
                             Hi[:].rearrange("p (q c) -> p q c", c=128),
                             Gv[:, :, 0, :], Gv[:, :, 1, :],
                             _b3(cs["itwAr"][:], rbb, 128),
                             _b3(cs["itwAi"][:], rbb, 128), tmpv)

                    # final: conv rows (f32 out for scratch)
                    convSB = dp.tile([128, rbb * 128], F32, tag="convSB",
                                     name="convSB")
                    for g in range(rbb // 4):
                        gsl = bass.ts(g, 512)
                        pc = pp.tile([128, 512], F32, tag="cc", name="cc")
                        nc.tensor.matmul(pc[:], lhsT=cs["V1Ar"][:],
                                         rhs=Hr[:, gsl], start=True, stop=False)
                        nc.tensor.matmul(pc[:], lhsT=cs["nV1Ai"][:],
                                         rhs=Hi[:, gsl], start=False, stop=True)
                        nc.scalar.copy(convSB[:, gsl], pc[:])

                    # batched scratch write: conv[0:14336] + elem 14336
                    nc.sync.dma_start(
                        scratch.ap()[r0:r0 + rbb, 0:14336]
                            .rearrange("r (a c) -> a r c", a=112),
                        convSB[0:112, :].rearrange("p (r c) -> p r c", c=128))
                    nc.sync.dma_start(
                        scratch.ap()[r0:r0 + rbb, 14336:14337]
                            .rearrange("r (c d) -> c r d", c=1),
                        convSB[112:113, :]
                            .rearrange("p (r c) -> p r c", c=128)[:, :, 0:1])

            # ---------------- C) 32K level ----------------
            with tc.tile_pool(name="p32", bufs=2) as dp:
                for b in range(nb2):
                    r0 = b * rb2
                    D2c = dp.tile([128, rb2 * 256], BF16, tag="D2c", name="D2c")
                    D2t = dp.tile([128, rb2 * 256], BF16, tag="D2t", name="D2t")
                    tgc = dp.tile([128, rb2 * 256], F32, tag="tgc", name="tgc")
                    tgt_ = dp.tile([128, rb2 * 256], F32, tag="tgt", name="tgt")
                    if b < 2:
                        # zero each double-buffer once; rows >= 56 and the
                        # row-56 gaps stay zero on reuse (never rewritten)
                        nc.scalar.memzero(D2c[:])
                        nc.scalar.memzero(D2t[:])
                        nc.scalar.memzero(tgc[:])
                        nc.scalar.memzero(tgt_[:])
                    nc.sync.dma_start(
                        tgc[0:56, :].rearrange("p (r c) -> p r c", c=256),
                        scratch.ap()[r0:r0 + rb2, 0:14336]
                            .rearrange("r (a c) -> a r c", a=56))
                    nc.sync.dma_start(
                        tgc[56:57, :].rearrange("p (r c) -> p r c", c=256)[:, :, 0:1],
                        scratch.ap()[r0:r0 + rb2, 14336:14337]
                            .rearrange("r (c d) -> c r d", c=1))
                    nc.sync.dma_start(
                        tgt_[0:56, :].rearrange("p (r c) -> p r c", c=256),
                        target.ap()[r0:r0 + rb2, 0:14336]
                            .rearrange("r (a c) -> a r c", a=56))
                    nc.sync.dma_start(
                        tgt_[56:57, :].rearrange("p (r c) -> p r c", c=256)[:, :, 0:1],
                        target.ap()[r0:r0 + rb2, 14336:14337]
                            .rearrange("r (c d) -> c r d", c=1))
                    # staging rows 0:56 fully rewritten; row 56 keeps zeros
                    # outside the q*256 slots (zeroed once before the loop)
                    nc.scalar.copy(D2c[0:57, :], tgc[0:57, :])
                    nc.scalar.copy(D2t[0:57, :], tgt_[0:57, :])

                    # stage 1: A[t2, f1] per q, both inputs; fused [Ar|Ai]
                    A2c = dp.tile([128, rb2 * 512], BF16, tag="A2c", name="A2c")
                    A2t = dp.tile([128, rb2 * 512], BF16, tag="A2t", name="A2t")
                    for q in range(rb2):
                        pac = pp.tile([128, 512], F32, tag="st1", name="st1")
                        pat = pp.tile([128, 512], F32, tag="st1", name="st1")
                        for cch in range(2):
                            dsl = slice(q * 256 + cch * 128,
                                        q * 256 + cch * 128 + 128)
                            nc.tensor.matmul(pac[:, bass.ts(cch, 256)],
                                             lhsT=D2c[0:57, dsl],
                                             rhs=cs["W1ri"][0:57, :],
                                             start=True, stop=True)
                            nc.tensor.matmul(pat[:, bass.ts(cch, 256)],
                                             lhsT=D2t[0:57, dsl],
                                             rhs=cs["W1ri"][0:57, :],
                                             start=True, stop=True)
                        nc.scalar.copy(A2c[:, bass.ts(q, 512)], pac[:])
                        nc.scalar.copy(A2t[:, bass.ts(q, 512)], pat[:])

                    # twiddle per t2-chunk: B = A * twB{c}
                    B2 = {}
                    tmp2 = dp.tile([128, rb2 * 128], BF16, tag="tmp2",
                                   name="tmp2")
                    tmp2v = tmp2[:].rearrange("p (q c) -> p q c", c=128)
                    for inp, At in (("c", A2c), ("t", A2t)):
                        Av = At[:].rearrange("p (q h s c) -> p q h s c",
                                             h=2, s=2, c=128)
                        for cch in range(2):
                            br = dp.tile([128, rb2 * 128], BF16,
                                         tag=f"B2r{cch}{inp}", name=f"B2r{cch}{inp}")
                            bi = dp.tile([128, rb2 * 128], BF16,
                                         tag=f"B2i{cch}{inp}", name=f"B2i{cch}{inp}")
                            _cmul_sb(nc,
                                     br[:].rearrange("p (q c) -> p q c", c=128),
                                     bi[:].rearrange("p (q c) -> p q c", c=128),
                                     Av[:, :, cch, 0, :], Av[:, :, cch, 1, :],
                                     _b3(cs[f"twBr{cch}"][:], rb2, 128),
                                     _b3(cs[f"twBi{cch}"][:], rb2, 128),
                                     tmp2v)
                            B2[(cch, inp)] = (br, bi)

                    # stage 2: Z[f2, f1]
                    Z2 = {}
                    for inp in ("c", "t"):
                        for f2c in range(2):
                            zr = dp.tile([128, rb2 * 128], BF16,
                                         tag=f"Z2r{inp}{f2c}", name=f"Z2r{inp}{f2c}")
                            zi = dp.tile([128, rb2 * 128], BF16,
                                         tag=f"Z2i{inp}{f2c}", name=f"Z2i{inp}{f2c}")
                            pzr = pp.tile([128, rb2 * 128], F32, tag="st2", name="st2")
                            pzi = pp.tile([128, rb2 * 128], F32, tag="st2", name="st2")
                            for t2c in range(2):
                                br, bi = B2[(t2c, inp)]
                                nc.tensor.matmul(pzr[:], lhsT=cs[f"W2Br{t2c}{f2c}"][:],
                                                 rhs=br[:], start=(t2c == 0), stop=False)
                                nc.tensor.matmul(pzr[:], lhsT=cs[f"nW2Bi{t2c}{f2c}"][:],
                                                 rhs=bi[:], start=False, stop=(t2c == 1))
                                nc.tensor.matmul(pzi[:], lhsT=cs[f"W2Bi{t2c}{f2c}"][:],
                                                 rhs=br[:], start=(t2c == 0), stop=False)
                                nc.tensor.matmul(pzi[:], lhsT=cs[f"W2Br{t2c}{f2c}"][:],
                                                 rhs=bi[:], start=False, stop=(t2c == 1))
                            nc.scalar.copy(zr[:], pzr[:])
                            nc.scalar.copy(zi[:], pzi[:])
                            Z2[(inp, f2c)] = (zr, zi)

                    # S = Zc * conj(Zt)  (bf16)
                    S2 = {}
                    tmpc = dp.tile([128, rb2 * 128], BF16, tag="tmpc", name="tmpc")
                    for f2c in range(2):
                        zcr, zci = Z2[("c", f2c)]
                        ztr, zti = Z2[("t", f2c)]
                        sr = dp.tile([128, rb2 * 128], BF16, tag=f"S2r{f2c}",
                                     name=f"S2r{f2c}")
                        si = dp.tile([128, rb2 * 128], BF16, tag=f"S2i{f2c}",
                                     name=f"S2i{f2c}")
                        nc.vector.tensor_tensor(sr[:], zcr[:], ztr[:], op=AT.mult)
                        nc.vector.tensor_tensor(tmpc[:], zci[:], zti[:], op=AT.mult)
                        nc.vector.tensor_tensor(sr[:], sr[:], tmpc[:], op=AT.add)
                        nc.vector.tensor_tensor(si[:], zci[:], ztr[:], op=AT.mult)
                        nc.vector.tensor_tensor(tmpc[:], zcr[:], zti[:], op=AT.mult)
                        nc.vector.tensor_tensor(si[:], si[:], tmpc[:], op=AT.subtract)
                        S2[f2c] = (sr, si)

                    # inverse stage 1: G[f1, t2] fused [Gr(256)|Gi(256)] per q
                    G2sb = dp.tile([128, rb2 * 512], BF16, tag="G2sb", name="G2sb")
                    for q in range(rb2):
                        pg = pp.tile([128, 512], F32, tag="inv", name="inv")
                        sl = bass.ts(q, 128)
                        for f2c in range(2):
                            sr, si = S2[f2c]
                            nc.tensor.matmul(pg[:], lhsT=sr[:, sl],
                                             rhs=cs[f"V2Bri{f2c}"][:],
                                             start=(f2c == 0), stop=False)
                            nc.tensor.matmul(pg[:], lhsT=si[:, sl],
                                             rhs=cs[f"V2Bnri{f2c}"][:],
                                             start=False, stop=(f2c == 1))
                        nc.scalar.copy(G2sb[:, bass.ts(q, 512)], pg[:])

                    # inverse twiddle: H = G * itwB (bf16)
                    H2r = dp.tile([128, rb2 * 256], BF16, tag="H2r", name="H2r")
                    H2i = dp.tile([128, rb2 * 256], BF16, tag="H2i", name="H2i")
                    G2v = G2sb[:].rearrange("p (q s c) -> p q s c", s=2, c=256)
                    tmph = dp.tile([128, rb2 * 256], BF16, tag="tmph", name="tmph")
                    _cmul_sb(nc,
                             H2r[:].rearrange("p (q c) -> p q c", c=256),
                             H2i[:].rearrange("p (q c) -> p q c", c=256),
                             G2v[:, :, 0, :], G2v[:, :, 1, :],
                             _b3(cs["itwBr"][:], rb2, 256),
                             _b3(cs["itwBi"][:], rb2, 256),
                             tmph[:].rearrange("p (q c) -> p q c", c=256))

                    # final: cc rows + mask + per-(t1,row) max
                    for g in range(rb2 // 2):
                        gsl = bass.ts(g, 512)
                        pcc = pp.tile([128, 512], F32, tag="cc", name="cc")
                        nc.tensor.matmul(pcc[:], lhsT=cs["V1Br"][:],
                                         rhs=H2r[:, gsl], start=True, stop=False)
                        nc.tensor.matmul(pcc[:], lhsT=cs["nV1Bi"][:],
                                         rhs=H2i[:, gsl], start=False, stop=True)
                        csl = slice((r0 + g * 2) * 256, (r0 + g * 2 + 2) * 256)
                        ccv = ccm_all[:, csl].rearrange("p (a b) -> p a b", b=256)
                        nc.vector.scalar_tensor_tensor(
                            ccv, pcc[:].rearrange("p (a b) -> p a b", b=256),
                            1.0, _b3(cs["maskB"][:], 2, 256),
                            op0=AT.bypass, op1=AT.add)
                        nc.vector.tensor_reduce(
                            allmax[:, r0 + g * 2:r0 + g * 2 + 2], ccv,
                            axis=AX.X, op=AT.max)

            # ---------------- D) argmax -> shifts -> loss ----------------
            with tc.tile_pool(name="amax", bufs=1) as dp:
                pt = pp.tile([rows, 128], F32, tag="st1", name="st1")
                nc.tensor.transpose(pt[:], allmax[:, 0:rows], cs["ident"][:])
                tmax = dp.tile([rows, 128], F32, tag="tmax", name="tmax")
                nc.scalar.copy(tmax[:], pt[:])
                rowmax = dp.tile([rows, 1], F32, tag="rowmax", name="rowmax")
                nc.vector.tensor_reduce(rowmax[:], tmax[:], axis=AX.X, op=AT.max)
                prm = pp.tile([1, rows], F32, tag="st2", name="st2")
                nc.tensor.transpose(prm[:], rowmax[:], cs["ident"][0:rows, 0:rows])
                rmT = dp.tile([1, rows], F32, tag="rmT", name="rmT")
                nc.scalar.copy(rmT[:], prm[:])
                pmb = pp.tile([128, rows], F32, tag="inv", name="inv")
                nc.tensor.matmul(pmb[:], lhsT=cs["ones1x128"][:], rhs=rmT[:],
                                 start=True, stop=True)
                Mb = dp.tile([128, rows], F32, tag="Mb", name="Mb")
                nc.scalar.copy(Mb[:], pmb[:])

                eqm = dp.tile([128, min(rows, 8) * 256], BF16, tag="eqm", name="eqm")
                selm = dp.tile([128, min(rows, 8) * 256], F32, tag="selm", name="selm")
                for bb in range(max(1, rows // 8)):
                    csl = bass.ts(bb, min(rows, 8) * 256)
                    nr8 = min(rows, 8)
                    mbb = Mb[:, bb * nr8:(bb + 1) * nr8]\
                        .rearrange("p (a b) -> p a b", b=1)\
                        .to_broadcast([128, nr8, 256])
                    ccv = ccm_all[:, csl].rearrange("p (a b) -> p a b", b=256)
                    nc.vector.tensor_tensor(
                        eqm[:].rearrange("p (a b) -> p a b", b=256),
                        ccv, mbb, op=AT.is_equal)
                    nc.vector.tensor_tensor(
                        selm[:].rearrange("p (a b) -> p a b", b=256),
                        eqm[:].rearrange("p (a b) -> p a b", b=256),
                        _b3(cs["shvB"][:], nr8, 256), op=AT.mult)
                    nc.vector.tensor_reduce(
                        allmin[:, bb * nr8:(bb + 1) * nr8],
                        selm[:].rearrange("p (a b) -> p a b", b=256),
                        axis=AX.X, op=AT.min)
                pt2 = pp.tile([rows, 128], F32, tag="cc", name="cc")
                nc.tensor.transpose(pt2[:], allmin[:, 0:rows], cs["ident"][:])
                tmin = dp.tile([rows, 128], F32, tag="tmin", name="tmin")
                nc.scalar.copy(tmin[:], pt2[:])
                nc.vector.tensor_reduce(shifts[:], tmin[:], axis=AX.X, op=AT.min)
                nc.vector.tensor_scalar_add(shifts[:], shifts[:], BIGL + float(START0))

                # start = (7040 + shift) mod 14337
                m1 = dp.tile([rows, 1], F32, tag="m1", name="m1")
                nc.vector.tensor_scalar(out=m1[:], in0=shifts[:], scalar1=0.0,
                                        scalar2=None, op0=AT.is_lt)
                nc.vector.scalar_tensor_tensor(
                    shifts[:], m1[:], float(CONV_LEN), shifts[:],
                    op0=AT.mult, op1=AT.add)
                nc.vector.tensor_scalar(out=m1[:], in0=shifts[:],
                                        scalar1=float(CONV_LEN), scalar2=None,
                                        op0=AT.is_ge)
                nc.vector.scalar_tensor_tensor(
                    shifts[:], m1[:], float(-CONV_LEN), shifts[:],
                    op0=AT.mult, op1=AT.add)

                # idx[r, i] = r*PITCH + (start + i) mod 14337
                idxf = dp.tile([rows, CROP], F32, tag="idxf", name="idxf")
                m2 = dp.tile([rows, CROP], F32, tag="m2", name="m2")
                nc.vector.tensor_tensor(idxf[:], cs["iota_i"][0:rows, :],
                                        shifts[:].to_broadcast([rows, CROP]),
                                        op=AT.add)
                nc.vector.tensor_scalar(out=m2[:], in0=idxf[:],
                                        scalar1=float(CONV_LEN), scalar2=None,
                                        op0=AT.is_ge)
                nc.vector.scalar_tensor_tensor(
                    idxf[:], m2[:], float(-CONV_LEN), idxf[:],
                    op0=AT.mult, op1=AT.add)
                nc.vector.tensor_tensor(
                    idxf[:], idxf[:],
                    cs["rowbase"][0:rows, :].to_broadcast([rows, CROP]),
                    op=AT.add)
                idxi = dp.tile([rows, CROP], I32, tag="idxi", name="idxi")
                nc.vector.tensor_copy(idxi[:], idxf[:])
                w = dp.tile([rows, CROP], F32, tag="wg", name="wg")
                nc.gpsimd.indirect_dma_start(
                    out=w[:], out_offset=None,
                    in_=scratch.ap().rearrange("r p -> (r p)").rearrange(
                        "(a b) -> a b", b=1),
                    in_offset=bass.IndirectOffsetOnAxis(ap=idxi[:], axis=0),
                )
                tw_ = dp.tile([rows, CROP], F32, tag="twin", name="twin")
                nc.sync.dma_start(tw_[:], target[:, START0:START0 + CROP])
                nc.vector.tensor_tensor(w[:], w[:], tw_[:], op=AT.subtract)
                convacc = dp.tile([rows, 1], F32, tag="convacc", name="convacc")
                nc.vector.scalar_tensor_tensor(
                    tw_[:], w[:], 1.0, w[:], op0=AT.bypass, op1=AT.mult,
                    accum_out=convacc[:])

                a0 = dp.tile([128, 1], F32, tag="a0", name="a0")
                nc.vector.tensor_reduce(a0[:], astf_acc[:], axis=AX.X, op=AT.add)
                psa = pp.tile([1, 1], F32, tag="st1", name="st1")
                nc.tensor.matmul(psa[:], lhsT=a0[:], rhs=cs["ones128"][:],
                                 start=True, stop=True)
                psc = pp.tile([1, 1], F32, tag="st2", name="st2")
                nc.tensor.matmul(psc[:], lhsT=convacc[:], rhs=cs["ones64"][0:rows, :],
                                 start=True, stop=True)
                nc.scalar.copy(outt[:, 0:1], psa[:])
                nc.scalar.copy(outt[:, 1:2], psc[:])
                nc.sync.dma_start(out[:], outt[:])

    nc.finalize()
    return nc, consts


_CACHE = {}


def get_built():
    if "nc" not in _CACHE:
        _CACHE["nc"] = build_nc()
    return _CACHE["nc"]


LAST_RESULT = {}


def kernel(pred_astf, true_astf, egf, target_waveform):
    import os
    import ml_dtypes
    from concourse.bass_utils import run_bass_kernel_spmd
    nc, consts = get_built()
    mmnames = _mm_const_names()
    consts = {k: (v.astype(ml_dtypes.bfloat16) if k in mmnames else v)
              for k, v in consts.items()}
    pred_astf = np.ascontiguousarray(np.asarray(pred_astf, np.float32))
    true_astf = np.ascontiguousarray(np.asarray(true_astf, np.float32))
    egf = np.ascontiguousarray(np.asarray(egf, np.float32))
    target_waveform = np.ascontiguousarray(
        np.asarray(target_waveform, np.float32))
    B = pred_astf.shape[0]
    per = B // NCORES
    in_maps = []
    for i in range(NCORES):
        sl = slice(i * per, (i + 1) * per)
        m = {"pred": pred_astf[sl], "true": true_astf[sl],
             "egf": egf[sl], "target": target_waveform[sl]}
        m.update(consts)
        in_maps.append(m)
    trace = os.environ.get("CONVALIGN_TRACE") == "1"
    res = run_bass_kernel_spmd(nc, in_maps, core_ids=list(range(NCORES)),
                               trace=trace)
    LAST_RESULT["res"] = res
    sums = np.stack([res.results[i]["out"][0] for i in range(NCORES)])
    loss_astf = np.float32(sums[:, 0].sum() / (B * L1))
    loss_conv = np.float32(sums[:, 1].sum() / (B * CROP))
    total = np.float32(loss_astf + loss_conv)
    return total, loss_astf, loss_conv
